# revision 1
# baseline (speedup 1.0000x reference)
"""Causal self-attention (B=4, T=2048, C=1024, 16 heads) on 8 trn2 NeuronCores.

Sharding: core c handles batch b = c//2 and an 8-head half hh = c%2
(tensor parallel over heads). Each core computes its heads' attention
output projected through its slice of w_proj rows; the host sums the two
partial projections per batch.

Device-side layout (per core):
  - QKV^T orientation: Q^T/K^T [feat, T] come straight out of the QKV
    matmul (lhsT = w chunk, rhs = x^T), V comes out in [T, feat] via the
    swapped orientation (lhsT = x^T chunk, rhs = w_v).
  - Scores are computed transposed, S^T[k, q], so softmax sums ride the
    A@V matmul as a ones-column appended to V (M=65).
  - exp has no max-subtraction (logits are N(0,1)-ish, |s|<40 -> safe in
    fp32), computed by ACT with the 1/sqrt(D) fused into its scale imm.
  - A and V' are float32r so the A@V matmul runs at full rate (N=512)
    with ~11-bit mantissa precision; QKV in bf16; proj in f32r.
"""
import os
import sys

if "/opt/trn_rl_repo" not in sys.path:
    sys.path.insert(0, "/opt/trn_rl_repo")
# The axon NTFF profiling hook is absent in this container; make sure the
# runner never takes the trace path (BASS_TRACE in the env would crash it).
os.environ.setdefault("BASS_NEVER_TRACE", "1")

import numpy as np
import ml_dtypes

B, T, C = 4, 2048, 1024
NH, D = 16, 64
P = 128
QC = 512           # q-chunk width
NQC = T // QC      # 4
NKB = T // P       # 16 k-blocks
GS = 2             # k-blocks per exp group (2 PSUM banks)
DH = 512           # per-core head feature width (8 heads * 64)

_CACHE = {}


def _build():
    import concourse.mybir as mybir
    import concourse.tile as tile
    from concourse import bacc

    f32 = mybir.dt.float32
    f32r = mybir.dt.float32r
    bf16 = mybir.dt.bfloat16
    MULT = mybir.AluOpType.mult
    EXP = mybir.ActivationFunctionType.Exp

    nc = bacc.Bacc(None, target_bir_lowering=False, debug=False)

    xt_d = nc.declare_dram_parameter("xt", [C, T], bf16, isOutput=False)
    wqk_d = nc.declare_dram_parameter("wqk", [C, 2 * DH], bf16, isOutput=False)
    wv_d = nc.declare_dram_parameter("wv", [C, DH], bf16, isOutput=False)
    wp_d = nc.declare_dram_parameter("wp", [DH, C], f32r, isOutput=False)
    dm_d = nc.declare_dram_parameter("dmask", [P, 1280], bf16, isOutput=False)
    id_d = nc.declare_dram_parameter("idq", [P, 2 * P], bf16, isOutput=False)
    out_d = nc.declare_dram_parameter("outT", [C, T], f32, isOutput=True)

    NCC = C // P  # 8 contraction chunks for QKV

    with tile.TileContext(nc) as tc:
        with (
            tc.tile_pool(name="pconst", bufs=1) as pconst,
            tc.tile_pool(name="pw", bufs=1) as pw,
            tc.tile_pool(name="px", bufs=1) as px,
            tc.tile_pool(name="pq", bufs=1) as pq,
            tc.tile_pool(name="pk", bufs=1) as pk,
            tc.tile_pool(name="pv", bufs=1) as pv,
            tc.tile_pool(name="pa", bufs=3) as pa,
            tc.tile_pool(name="psml", bufs=2) as psml,
            tc.tile_pool(name="posb", bufs=1) as posb,
            tc.tile_pool(name="psS", bufs=2, space="PSUM") as psS,
            tc.tile_pool(name="psX", bufs=4, space="PSUM") as psX,
        ):
            # ---- constants / weights / full-resident x^T ----
            dm_t = pconst.tile([P, 1280], bf16, name="dm")
            DMOFF = (0, 128, 384, 768)
            id_t = pconst.tile([P, 2 * P], bf16, name="idt")
            ones_c = pconst.tile([P, 8, 1], f32, name="ones_c")
            ones_r = pconst.tile([P, P], f32r, name="ones_r")

            x_t = []
            wqk_t = []
            wv_t = []
            for i in range(NCC):
                xt_ = px.tile([P, T], bf16, tag=f"x{i}", name=f"x{i}")
                nc.sync.dma_start(xt_[:], xt_d[P * i : P * (i + 1), :])
                x_t.append(xt_)
                t_ = pw.tile([P, 2 * DH], bf16, tag=f"wqk{i}", name=f"wqk{i}")
                nc.sync.dma_start(t_[:], wqk_d[P * i : P * (i + 1), :])
                wqk_t.append(t_)
                v_ = pw.tile([P, DH], bf16, tag=f"wv{i}", name=f"wv{i}")
                nc.sync.dma_start(v_[:], wv_d[P * i : P * (i + 1), :])
                wv_t.append(v_)
            nc.sync.dma_start(dm_t[:], dm_d[:])
            nc.sync.dma_start(id_t[:], id_d[:])
            nc.vector.memset(ones_c[:], 1.0)
            nc.vector.tensor_copy(ones_r[64:65, :], id_t[64:65, P : 2 * P])
            wp_t = []
            for i in range(4):
                t_ = pw.tile([P, C], f32r, tag=f"wp{i}", name=f"wp{i}")
                nc.sync.dma_start(t_[:], wp_d[P * i : P * (i + 1), :])
                wp_t.append(t_)

            # ---- persistent stores ----
            # K^T bf16 per (j-block, qc): [128 feat, 512 k-cols]
            k_sb = [
                [pk.tile([P, QC], bf16, tag=f"k{j}_{m}", name=f"k{j}_{m}")
                 for m in range(NQC)]
                for j in range(4)
            ]
            q_sb = [
                [pq.tile([P, QC], bf16, tag=f"q{j}_{m}", name=f"q{j}_{m}")
                 for m in range(NQC)]
                for j in range(4)
            ]
            # V' f32r per k-block: [128 k, 8 heads, 65] (col 64 = ones)
            vp = [pv.tile([P, 8, 65], f32r, tag=f"vp{kb}", name=f"vp{kb}")
                  for kb in range(NKB)]
            # O^T f32r per (cin-chunk, qc): [128 feat, 512 q]
            o_sb = [
                [posb.tile([P, QC], f32r, tag=f"o{i}_{m}", name=f"o{i}_{m}")
                 for m in range(NQC)]
                for i in range(4)
            ]

            # ---- QKV phase: j-outer so each weight LDW serves 4 q-chunks ----
            for j in range(8):
                if j % 2 == 0:
                    pps = [psX.tile([P, QC], f32, tag="pp", name=f"qkps{j}_{m}")
                           for m in range(NQC)]
                else:
                    ppw = [psS.tile([P, GS * QC], f32, tag="sg",
                                    name=f"qkps{j}_{m}") for m in range(2)]
                    pps = [ppw[0][:, 0:QC], ppw[0][:, QC:2 * QC],
                           ppw[1][:, 0:QC], ppw[1][:, QC:2 * QC]]
                for i in range(NCC):
                    for m in range(NQC):
                        nc.tensor.matmul(
                            pps[m][:],
                            wqk_t[i][:, P * j : P * (j + 1)],
                            x_t[i][:, QC * m : QC * (m + 1)],
                            start=(i == 0),
                            stop=(i == NCC - 1),
                        )
                for m in range(NQC):
                    if j < 4:
                        nc.vector.tensor_copy(q_sb[j][m][:], pps[m][:])
                    else:
                        nc.vector.tensor_copy(k_sb[j - 4][m][:], pps[m][:])
            for kb in range(NKB):
                if kb % 2 == 0:
                    pp = psX.tile([P, QC], f32, tag="pp", name=f"vps{kb}")
                else:
                    ppw = psS.tile([P, GS * QC], f32, tag="sg", name=f"vps{kb}")
                    pp = ppw[:, 0:QC]
                for i in range(NCC):
                    nc.tensor.matmul(
                        pp[:],
                        x_t[i][:, P * kb : P * (kb + 1)],
                        wv_t[i][:],
                        start=(i == 0),
                        stop=(i == NCC - 1),
                    )
                nc.vector.tensor_copy(vp[kb][:, :, 64:65], ones_c[:])
                nc.vector.tensor_copy(
                    vp[kb][:, :, 0:64],
                    pp[:].rearrange("p (h d) -> p h d", d=64),
                )

            for n in range(NQC):
                # ---- attention for all heads at q-chunk n ----                # ---- attention for all heads at q-chunk n ----
                nkb = 4 * (n + 1)
                groups = [
                    list(range(g, min(g + GS, nkb))) for g in range(0, nkb, GS)
                ]
                for jq in range(4):
                    po = {}
                    for half in (0, 1):
                        h = 2 * jq + half
                        po[half] = psX.tile([P, QC], f32, tag="pp",
                                            name=f"po{h}_{n}")
                    sgt = {}
                    for gi, grp in enumerate(groups):
                        for half in (0, 1):
                            h = 2 * jq + half
                            r0 = 64 * half
                            sg = psS.tile([P, GS * QC], f32, tag="sg",
                                          name=f"sg{h}_{n}_{gi}")
                            sgt[half] = sg
                            for t_i, kb in enumerate(grp):
                                dst = sg[:, QC * t_i : QC * (t_i + 1)]
                                lhs_k = k_sb[jq][kb // 4][
                                    r0 : r0 + 64,
                                    P * (kb % 4) : P * (kb % 4 + 1),
                                ]
                                rhs_q = q_sb[jq][n][r0 : r0 + 64, :]
                                diag = kb >= 4 * n
                                if diag:
                                    tt = kb - 4 * n
                                    wmask = P * (tt + 1)
                                    nc.tensor.matmul(
                                        dst[:, 0:wmask], id_t[:, 0:P],
                                        dm_t[:, DMOFF[tt] : DMOFF[tt] + wmask],
                                        start=True, stop=True,
                                    )
                                    nc.tensor.matmul(
                                        dst[:, P * tt : QC],
                                        lhs_k[:, :],
                                        rhs_q[:, P * tt : QC],
                                        start=False, stop=True,
                                    )
                                else:
                                    nc.tensor.matmul(
                                        dst, lhs_k, rhs_q,
                                        start=True, stop=True,
                                    )
                        ats = {}
                        w = QC * len(grp)
                        for half in (0, 1):
                            h = 2 * jq + half
                            at = pa.tile([P, GS * QC], f32r, tag="at",
                                         name=f"at{h}_{n}_{gi}")
                            nc.scalar.activation(at[:, :w], sgt[half][:, :w],
                                                 EXP, scale=0.125)
                            ats[half] = at
                        for t_i, kb in enumerate(grp):
                            for half in (0, 1):
                                h = 2 * jq + half
                                nc.tensor.matmul(
                                    po[half][0:65, :],
                                    vp[kb][:, h, :],
                                    ats[half][:, QC * t_i : QC * (t_i + 1)],
                                    start=(kb == 0),
                                    stop=(kb == nkb - 1),
                                )
                    sums_rs, spss, rrs = {}, {}, {}
                    for half in (0, 1):
                        h = 2 * jq + half
                        sums_r = psml.tile([P, QC], f32r, tag="rb",
                                           name=f"sums{h}_{n}")
                        nc.vector.tensor_copy(sums_r[64:65, :],
                                              po[half][64:65, :])
                        sums_rs[half] = sums_r
                    for half in (0, 1):
                        h = 2 * jq + half
                        sps = psX.tile([P, QC], f32, tag="pp",
                                       name=f"sps{h}_{n}")
                        nc.tensor.matmul(sps[:], ones_r[64:65, :],
                                         sums_rs[half][64:65, :],
                                         start=True, stop=True)
                        spss[half] = sps
                    for half in (0, 1):
                        h = 2 * jq + half
                        rr = psml.tile([P, QC], f32, tag="rr",
                                       name=f"rr{h}_{n}")
                        nc.vector.reciprocal_approx_fast(rr[:], spss[half][:])
                        rrs[half] = rr
                    for half in (0, 1):
                        h = 2 * jq + half
                        if half == 0:
                            nc.vector.tensor_tensor(
                                o_sb[jq][n][0:64, :],
                                po[half][0:64, :],
                                rrs[half][0:64, :],
                                MULT,
                            )
                        else:
                            onorm = psml.tile([P, QC], f32r, tag="onorm",
                                              name=f"onorm{h}_{n}")
                            nc.vector.tensor_tensor(
                                onorm[0:64, :],
                                po[half][0:64, :],
                                rrs[half][0:64, :],
                                MULT,
                            )
                            nc.sync.dma_start(
                                o_sb[jq][n][64:128, :], onorm[0:64, :]
                            )

            # ---- output projection ----
            for j2 in range(8):
                for n in range(NQC):
                    k2 = j2 * NQC + n
                    if k2 % 2 == 0:
                        pp = psX.tile([P, QC], f32, tag="pp", name=f"pj{j2}_{n}")
                    else:
                        ppw = psS.tile([P, GS * QC], f32, tag="sg",
                                       name=f"pj{j2}_{n}")
                        pp = ppw[:, 0:QC]
                    for i2 in range(4):
                        nc.tensor.matmul(
                            pp[:],
                            wp_t[i2][:, P * j2 : P * (j2 + 1)],
                            o_sb[i2][n][:],
                            start=(i2 == 0),
                            stop=(i2 == 3),
                        )
                    oo = psml.tile([P, QC], f32, tag=f"oo{k2 % 2}",
                                   name=f"oo{j2}_{n}")
                    if k2 % 2 == 0:
                        nc.vector.tensor_copy(oo[:], pp[:])
                    else:
                        nc.scalar.copy(oo[:], pp[:])
                    nc.sync.dma_start(
                        out_d[P * j2 : P * (j2 + 1), QC * n : QC * (n + 1)],
                        oo[:],
                    )

    nc.compile()
    return nc


def _get_nc():
    if "nc" not in _CACHE:
        _CACHE["nc"] = _build()
    return _CACHE["nc"]


def _make_dmask():
    # packed additive masks, window t has width 128*(t+1):
    # dm[r, off_t + c] = -1e4 if c < 128*t + r else 0
    offs = (0, 128, 384, 768)
    dm = np.zeros((P, 1280), np.float32)
    for t in range(4):
        w = 128 * (t + 1)
        for r in range(P):
            dm[r, offs[t] : offs[t] + min(128 * t + r, w)] = -1e4
    return dm.astype(ml_dtypes.bfloat16)


def _in_maps(x, w_qkv, w_proj):
    bf = ml_dtypes.bfloat16
    dm = _make_dmask()
    idq = np.concatenate([np.eye(P, dtype=np.float32),
                          np.ones((P, P), np.float32)], axis=1).astype(bf)
    maps = []
    for c in range(8):
        b, hh = divmod(c, 2)
        xT = np.ascontiguousarray(x[b].T).astype(bf)
        qcols = w_qkv[:, DH * hh : DH * hh + DH]
        kcols = w_qkv[:, C + DH * hh : C + DH * hh + DH]
        vcols = w_qkv[:, 2 * C + DH * hh : 2 * C + DH * hh + DH]
        maps.append({
            "xt": xT,
            "wqk": np.concatenate([qcols, kcols], axis=1).astype(bf),
            "wv": np.ascontiguousarray(vcols).astype(bf),
            "wp": np.ascontiguousarray(w_proj[DH * hh : DH * hh + DH, :],
                                       dtype=np.float32),
            "dmask": dm,
            "idq": idq,
        })
    return maps


def _run(x, w_qkv, w_proj, trace=False):
    from concourse.bass_utils import run_bass_kernel_spmd

    nc = _get_nc()
    maps = _in_maps(x, w_qkv, w_proj)
    res = run_bass_kernel_spmd(nc, maps, list(range(8)), trace=trace)
    out = np.empty((B, T, C), np.float32)
    for b in range(B):
        out[b] = res.results[2 * b]["outT"].T + res.results[2 * b + 1]["outT"].T
    return out, res


def kernel(**inputs):
    x = np.asarray(inputs["x"], dtype=np.float32)
    w_qkv = np.asarray(inputs["w_qkv"], dtype=np.float32)
    w_proj = np.asarray(inputs["w_proj"], dtype=np.float32)
    out, _ = _run(x, w_qkv, w_proj, trace=False)
    return out






# revision 15
# speedup vs baseline: 1.0231x; 1.0231x over previous
"""Causal self-attention (B=4, T=2048, C=1024, 16 heads) on 8 trn2 NeuronCores.

Sharding: core c handles batch b = c//2 and an 8-head half hh = c%2
(tensor parallel over heads). Each core computes its heads' attention
output projected through its slice of w_proj rows; the host sums the two
partial projections per batch.

Device-side layout (per core):
  - QKV^T orientation: Q^T/K^T [feat, T] come straight out of the QKV
    matmul (lhsT = w chunk, rhs = x^T); V comes out in [T, feat] via the
    swapped orientation (lhsT = x^T chunk, rhs = w_v).
  - Scores are computed transposed, S^T[k, q]; softmax sums ride the
    A@V matmul as a ones-column appended to V (M=65).
  - Causality: strictly-below-diagonal k-blocks are computed full-width;
    the 4 diagonal blocks get a 128-wide additive-mask triangle (PE
    matmul: eye @ tri) and column-restricted score/exp-consume/A@V, so
    no PE rows are spent above the diagonal.
  - exp has no max-subtraction (logits are N(0,1)-ish, safe in fp32),
    computed by ACT with the 1/sqrt(D) fused into its scale imm.
  - A and V' are bf16 so restricted (narrow) A@V matmuls still run at
    1 cycle/row; Q/K bf16; proj weights and O in f32r.
  - The attention stream for chunk n is software-pipelined with filler
    matmuls (QKV for chunk n+1, projection of earlier chunks) so the PE
    array keeps working while ACT computes exp.
"""
import os
import sys
from collections import deque

if "/opt/trn_rl_repo" not in sys.path:
    sys.path.insert(0, "/opt/trn_rl_repo")
# The axon NTFF profiling hook is absent in this container; make sure the
# runner never takes the trace path (BASS_TRACE in the env would crash it).
os.environ.setdefault("BASS_NEVER_TRACE", "1")

import numpy as np
import ml_dtypes

B, T, C = 4, 2048, 1024
NH, D = 16, 64
P = 128
QC = 512           # q-chunk width
NQC = T // QC      # 4
NKB = T // P       # 16 k-blocks
GS = 2             # k-blocks per exp group
DH = 512           # per-core head feature width (8 heads * 64)
NCC = C // P       # 8 contraction chunks for QKV

_CACHE = {}


def _build():
    import concourse.mybir as mybir
    import concourse.tile as tile
    from concourse import bacc

    f32 = mybir.dt.float32
    f32r = mybir.dt.float32r
    bf16 = mybir.dt.bfloat16
    MULT = mybir.AluOpType.mult
    EXP = mybir.ActivationFunctionType.Exp

    nc = bacc.Bacc(None, target_bir_lowering=False, debug=False)

    xt_d = nc.declare_dram_parameter("xt", [C, T], bf16, isOutput=False)
    wqk_d = nc.declare_dram_parameter("wqk", [C, 2 * DH], bf16, isOutput=False)
    wv_d = nc.declare_dram_parameter("wv", [C, DH], bf16, isOutput=False)
    wp_d = nc.declare_dram_parameter("wp", [DH, C], f32r, isOutput=False)
    tri_d = nc.declare_dram_parameter("tri", [P, P], bf16, isOutput=False)
    id_d = nc.declare_dram_parameter("idq", [P, 2 * P], bf16, isOutput=False)
    out_d = nc.declare_dram_parameter("outT", [C, T], f32, isOutput=True)

    with tile.TileContext(nc) as tc:
        with (
            tc.tile_pool(name="pconst", bufs=1) as pconst,
            tc.tile_pool(name="pw", bufs=1) as pw,
            tc.tile_pool(name="px", bufs=1) as px,
            tc.tile_pool(name="pq", bufs=1) as pq,
            tc.tile_pool(name="pk", bufs=1) as pk,
            tc.tile_pool(name="pv", bufs=1) as pv,
            tc.tile_pool(name="pa", bufs=3) as pa,
            tc.tile_pool(name="psb", bufs=2) as psb,
            tc.tile_pool(name="posb", bufs=1) as posb,
            tc.tile_pool(name="psS", bufs=2, space="PSUM") as psS,
            tc.tile_pool(name="psX", bufs=2, space="PSUM") as psX,
        ):
            # ---- constants / weights / full-resident x^T ----
            tri_t = pconst.tile([P, P], bf16, name="tri")
            id_t = pconst.tile([P, 2 * P], bf16, name="idt")
            ones_f = pconst.tile([P, 64], f32r, name="ones_f")
            ones_c = pconst.tile([P, 8, 1], f32, name="ones_c")

            x_t = []
            wqk_t = []
            wv_t = []
            for i in range(NCC):
                xt_ = px.tile([P, T], bf16, tag=f"x{i}", name=f"x{i}")
                nc.sync.dma_start(xt_[:], xt_d[P * i : P * (i + 1), :])
                x_t.append(xt_)
                t_ = pw.tile([P, 2 * DH], bf16, tag=f"wqk{i}", name=f"wqk{i}")
                nc.sync.dma_start(t_[:], wqk_d[P * i : P * (i + 1), :])
                wqk_t.append(t_)
                v_ = pw.tile([P, DH], bf16, tag=f"wv{i}", name=f"wv{i}")
                nc.sync.dma_start(v_[:], wv_d[P * i : P * (i + 1), :])
                wv_t.append(v_)
            nc.sync.dma_start(tri_t[:], tri_d[:])
            nc.sync.dma_start(id_t[:], id_d[:])
            nc.vector.memset(ones_c[:], 1.0)
            nc.vector.tensor_copy(ones_f[64:65, :], id_t[64:65, P : P + 64])
            wp_t = []
            for i in range(4):
                t_ = pw.tile([P, C], f32r, tag=f"wp{i}", name=f"wp{i}")
                nc.sync.dma_start(t_[:], wp_d[P * i : P * (i + 1), :])
                wp_t.append(t_)

            # ---- persistent stores ----
            # K^T / Q^T bf16 per (feat-pair hp, q-window m): [128 feat, 512]
            k_sb = [
                [pk.tile([P, QC], bf16, tag=f"k{j}_{m}", name=f"k{j}_{m}")
                 for m in range(NQC)]
                for j in range(4)
            ]
            q_sb = [
                [pq.tile([P, QC], bf16, tag=f"q{j}_{m}", name=f"q{j}_{m}")
                 for m in range(NQC)]
                for j in range(4)
            ]
            # V' bf16 per k-block: [128 k, 8 heads, 65] (col 64 = ones)
            vp = [pv.tile([P, 8, 65], bf16, tag=f"vp{kb}", name=f"vp{kb}")
                  for kb in range(NKB)]
            # O^T f32r per (feat-pair hp, q-chunk n): [128 feat, 512 q]
            o_sb = [
                [posb.tile([P, QC], f32r, tag=f"o{i}_{m}", name=f"o{i}_{m}")
                 for m in range(NQC)]
                for i in range(4)
            ]

            # ---- filler units (software pipelining) ----
            def u_qk(kind, j, m):
                def emit():
                    pp = psX.tile([P, QC], f32, tag="fl",
                                  name=f"{kind}ps{j}_{m}")
                    off = 0 if kind == "q" else DH
                    for i in range(NCC):
                        nc.tensor.matmul(
                            pp[:],
                            wqk_t[i][:, off + P * j : off + P * (j + 1)],
                            x_t[i][:, QC * m : QC * (m + 1)],
                            start=(i == 0),
                            stop=(i == NCC - 1),
                        )
                    dst = q_sb if kind == "q" else k_sb
                    nc.vector.tensor_copy(dst[j][m][:], pp[:])
                return emit

            def u_v(kb):
                def emit():
                    pp = psX.tile([P, QC], f32, tag="fl", name=f"vps{kb}")
                    for i in range(NCC):
                        nc.tensor.matmul(
                            pp[:],
                            x_t[i][:, P * kb : P * (kb + 1)],
                            wv_t[i][:],
                            start=(i == 0),
                            stop=(i == NCC - 1),
                        )
                    nc.vector.tensor_copy(
                        vp[kb][:, :, 0:64],
                        pp[:].rearrange("p (h d) -> p h d", d=64),
                    )
                    nc.vector.tensor_copy(vp[kb][:, :, 64:65], ones_c[:])
                return emit

            def u_proj(n, j2):
                def emit():
                    pp = psX.tile([P, QC], f32, tag="fl", name=f"pj{j2}_{n}")
                    for i2 in range(4):
                        nc.tensor.matmul(
                            pp[:],
                            wp_t[i2][:, P * j2 : P * (j2 + 1)],
                            o_sb[i2][n][:],
                            start=(i2 == 0),
                            stop=(i2 == 3),
                        )
                    oo = psb.tile([P, QC], f32, tag="oo", name=f"oo{j2}_{n}")
                    nc.vector.tensor_copy(oo[:], pp[:])
                    nc.sync.dma_start(
                        out_d[P * j2 : P * (j2 + 1), QC * n : QC * (n + 1)],
                        oo[:],
                    )
                return emit

            def qkv_units(m):
                us = [u_qk("k", 0, m), u_qk("q", 0, m)]
                us += [u_v(4 * m + t) for t in range(4)]
                for j in range(1, 4):
                    us += [u_qk("k", j, m), u_qk("q", j, m)]
                return us

            # ---- prologue: QKV for chunk 0 ----
            for u in qkv_units(0):
                u()

            # ---- attention chunks with wedged fillers ----
            FQ = deque()
            pace = {"seen": 0, "emitted": 0, "points": 1, "units": 0}

            def phase(units, points):
                FQ.extend(units)
                pace["seen"] = 0
                pace["emitted"] = 0
                pace["points"] = max(points, 1)
                pace["units"] = len(FQ)

            def wedge():
                pace["seen"] += 1
                while (FQ and pace["emitted"] * pace["points"]
                       < pace["seen"] * pace["units"]):
                    FQ.popleft()()
                    pace["emitted"] += 1

            for n in range(NQC):
                nkb = 4 * (n + 1)
                ngrp = nkb // GS
                if n < 3:
                    phase(qkv_units(n + 1), 8 * (ngrp + 2))
                else:
                    phase([u_proj(np_, j2) for np_ in range(3)
                           for j2 in range(8)], 8 * (ngrp + 2))

                for h in range(8):
                    hp, par = divmod(h, 2)
                    r0 = 64 * par
                    po = psX.tile([P, QC], f32, tag="po", name=f"po{h}_{n}")
                    for gi in range(ngrp):
                        grp = [GS * gi, GS * gi + 1]
                        sg = psS.tile([P, GS * QC], f32, tag="sg",
                                      name=f"sg{h}_{n}_{gi}")
                        for t_i, kb in enumerate(grp):
                            lhs_k = k_sb[hp][kb // 4][
                                r0 : r0 + 64,
                                P * (kb % 4) : P * (kb % 4 + 1),
                            ]
                            if kb >= 4 * n:
                                tt = kb - 4 * n
                                c0 = P * tt
                                nc.tensor.matmul(
                                    sg[:, QC * t_i + c0 : QC * t_i + c0 + P],
                                    id_t[:, 0:P], tri_t[:],
                                    start=True, stop=True,
                                )
                                nc.tensor.matmul(
                                    sg[:, QC * t_i + c0 : QC * (t_i + 1)],
                                    lhs_k,
                                    q_sb[hp][n][r0 : r0 + 64, c0:QC],
                                    start=False, stop=True,
                                )
                            else:
                                nc.tensor.matmul(
                                    sg[:, QC * t_i : QC * (t_i + 1)],
                                    lhs_k,
                                    q_sb[hp][n][r0 : r0 + 64, :],
                                    start=True, stop=True,
                                )
                        at = pa.tile([P, GS * QC], bf16, tag="at",
                                     name=f"at{h}_{n}_{gi}")
                        nc.scalar.activation(at[:], sg[:], EXP, scale=0.125)
                        wedge()
                        for t_i, kb in enumerate(grp):
                            c0 = P * (kb - 4 * n) if kb >= 4 * n else 0
                            nc.tensor.matmul(
                                po[0:65, c0:QC],
                                vp[kb][:, h, :],
                                at[:, QC * t_i + c0 : QC * (t_i + 1)],
                                start=(kb == 0),
                                stop=(kb >= 4 * n),
                            )
                    # ---- normalize: sums -> replicate -> recip -> mult ----
                    sums = psb.tile([P, QC], f32r, tag="sm", name=f"sm{h}_{n}")
                    nc.vector.tensor_copy(sums[64:65, :], po[64:65, :])
                    wedge()
                    rep = psX.tile([P, QC], f32, tag="fl", name=f"rep{h}_{n}")
                    rr = psb.tile([P, QC], f32, tag="rr", name=f"rr{h}_{n}")
                    nc.tensor.matmul(rep[0:64, :], ones_f[64:65, :],
                                     sums[64:65, :], start=True, stop=True)
                    nc.vector.reciprocal_approx_fast(rr[0:64, :], rep[0:64, :])
                    if par == 0:
                        nc.vector.tensor_tensor(o_sb[hp][n][0:64, :],
                                                po[0:64, :], rr[0:64, :], MULT)
                    else:
                        onorm = psb.tile([P, QC], f32r, tag="on",
                                         name=f"on{h}_{n}")
                        nc.vector.tensor_tensor(onorm[0:64, :], po[0:64, :],
                                                rr[0:64, :], MULT)
                        nc.sync.dma_start(o_sb[hp][n][64:128, :],
                                          onorm[0:64, :])
                    wedge()

            # drain any leftover fillers, then final projection
            while FQ:
                FQ.popleft()()
            for j2 in range(8):
                u_proj(3, j2)()

    nc.compile()
    return nc


def _get_nc():
    if "nc" not in _CACHE:
        _CACHE["nc"] = _build()
    return _CACHE["nc"]


def _make_tri():
    # additive causal mask for a 128x128 diagonal block of S^T[k, q]:
    # tri[r, c] = -1e4 where q-col c < k-row r (strictly above diagonal)
    tri = np.zeros((P, P), np.float32)
    for r in range(P):
        tri[r, :r] = -1e4
    return tri.astype(ml_dtypes.bfloat16)


def _in_maps(x, w_qkv, w_proj):
    bf = ml_dtypes.bfloat16
    tri = _make_tri()
    idq = np.concatenate([np.eye(P, dtype=np.float32),
                          np.ones((P, P), np.float32)], axis=1).astype(bf)
    maps = []
    for c in range(8):
        b, hh = divmod(c, 2)
        xT = np.ascontiguousarray(x[b].T).astype(bf)
        qcols = w_qkv[:, DH * hh : DH * hh + DH]
        kcols = w_qkv[:, C + DH * hh : C + DH * hh + DH]
        vcols = w_qkv[:, 2 * C + DH * hh : 2 * C + DH * hh + DH]
        maps.append({
            "xt": xT,
            "wqk": np.concatenate([qcols, kcols], axis=1).astype(bf),
            "wv": np.ascontiguousarray(vcols).astype(bf),
            "wp": np.ascontiguousarray(w_proj[DH * hh : DH * hh + DH, :],
                                       dtype=np.float32),
            "tri": tri,
            "idq": idq,
        })
    return maps


def _run(x, w_qkv, w_proj, trace=False):
    from concourse.bass_utils import run_bass_kernel_spmd

    nc = _get_nc()
    maps = _in_maps(x, w_qkv, w_proj)
    res = run_bass_kernel_spmd(nc, maps, list(range(8)), trace=trace)
    out = np.empty((B, T, C), np.float32)
    for b in range(B):
        out[b] = res.results[2 * b]["outT"].T + res.results[2 * b + 1]["outT"].T
    return out, res


def kernel(**inputs):
    x = np.asarray(inputs["x"], dtype=np.float32)
    w_qkv = np.asarray(inputs["w_qkv"], dtype=np.float32)
    w_proj = np.asarray(inputs["w_proj"], dtype=np.float32)
    out, _ = _run(x, w_qkv, w_proj, trace=False)
    return out


# revision 22
# speedup vs baseline: 1.0436x; 1.0201x over previous
"""Causal self-attention (B=4, T=2048, C=1024, 16 heads) on 8 trn2 NeuronCores.

Sharding: core c handles batch b = c//2 and an 8-head half hh = c%2
(tensor parallel over heads). Each core computes its heads' attention
output projected through its slice of w_proj rows; the host sums the two
partial projections per batch.

Device-side layout (per core):
  - QKV^T orientation: Q^T/K^T [feat, T] come straight out of the QKV
    matmul (lhsT = w chunk, rhs = x^T); V comes out in [T, feat] via the
    swapped orientation (lhsT = x^T chunk, rhs = w_v).
  - Scores are computed transposed, S^T[k, q]; softmax sums ride the
    A@V matmul as a ones-column appended to V (M=65).
  - Causality: strictly-below-diagonal k-blocks are computed full-width;
    the 4 diagonal blocks get a 128-wide additive-mask triangle (PE
    matmul: eye @ tri) and column-restricted score/exp-consume/A@V, so
    no PE rows are spent above the diagonal.
  - exp has no max-subtraction (logits are N(0,1)-ish, safe in fp32),
    computed by ACT with the 1/sqrt(D) fused into its scale imm.
  - A and V' are bf16 so restricted (narrow) A@V matmuls still run at
    1 cycle/row; Q/K bf16; proj weights and O in f32r.
  - The attention stream for chunk n is software-pipelined with filler
    matmuls (QKV for chunk n+1, projection of earlier chunks) so the PE
    array keeps working while ACT computes exp.
"""
import os
import sys
from collections import deque

if "/opt/trn_rl_repo" not in sys.path:
    sys.path.insert(0, "/opt/trn_rl_repo")
# The axon NTFF profiling hook is absent in this container; make sure the
# runner never takes the trace path (BASS_TRACE in the env would crash it).
os.environ.setdefault("BASS_NEVER_TRACE", "1")

import numpy as np
import ml_dtypes

B, T, C = 4, 2048, 1024
NH, D = 16, 64
P = 128
QC = 512           # q-chunk width
NQC = T // QC      # 4
NKB = T // P       # 16 k-blocks
GS = 2             # k-blocks per exp group
DH = 512           # per-core head feature width (8 heads * 64)
NCC = C // P       # 8 contraction chunks for QKV

_CACHE = {}


def _build():
    import concourse.mybir as mybir
    import concourse.tile as tile
    from concourse import bacc

    f32 = mybir.dt.float32
    f32r = mybir.dt.float32r
    bf16 = mybir.dt.bfloat16
    MULT = mybir.AluOpType.mult
    EXP = mybir.ActivationFunctionType.Exp

    nc = bacc.Bacc(None, target_bir_lowering=False, debug=False)

    xt_d = nc.declare_dram_parameter("xt", [C, T], bf16, isOutput=False)
    wqk_d = nc.declare_dram_parameter("wqk", [C, 2 * DH], bf16, isOutput=False)
    wv_d = nc.declare_dram_parameter("wv", [C, DH], bf16, isOutput=False)
    wp_d = nc.declare_dram_parameter("wp", [DH, C], f32r, isOutput=False)
    tri_d = nc.declare_dram_parameter("tri", [P, P], bf16, isOutput=False)
    id_d = nc.declare_dram_parameter("idq", [P, 2 * P], bf16, isOutput=False)
    out_d = nc.declare_dram_parameter("outT", [C, T], f32, isOutput=True)

    with tile.TileContext(nc) as tc:
        with (
            tc.tile_pool(name="pconst", bufs=1) as pconst,
            tc.tile_pool(name="pw", bufs=1) as pw,
            tc.tile_pool(name="px", bufs=1) as px,
            tc.tile_pool(name="pq", bufs=1) as pq,
            tc.tile_pool(name="pk", bufs=1) as pk,
            tc.tile_pool(name="pv", bufs=1) as pv,
            tc.tile_pool(name="pa", bufs=3) as pa,
            tc.tile_pool(name="psb", bufs=2) as psb,
            tc.tile_pool(name="posb", bufs=1) as posb,
            tc.tile_pool(name="psS", bufs=2, space="PSUM") as psS,
            tc.tile_pool(name="psX", bufs=2, space="PSUM") as psX,
        ):
            # ---- constants / weights / full-resident x^T ----
            tri_t = pconst.tile([P, P], bf16, name="tri")
            id_t = pconst.tile([P, 2 * P], bf16, name="idt")
            ones_f = pconst.tile([P, 64], f32r, name="ones_f")
            ones_c = pconst.tile([P, 8, 1], f32, name="ones_c")

            # x^T chunks stream on the SP HWDGE queue; weights go column-block
            # by column-block on the ACT HWDGE queue so the first QKV units
            # unblock after one small DMA each instead of eight full-chunk
            # loads.
            x_t = []
            for i in range(NCC):
                xt_ = px.tile([P, T], bf16, tag=f"x{i}", name=f"x{i}")
                nc.sync.dma_start(xt_[:], xt_d[P * i : P * (i + 1), :])
                x_t.append(xt_)

            wqk_t = pw.tile([P, NCC, 2 * DH], bf16, tag="wqk", name="wqk")
            wv_t = pw.tile([P, NCC, DH], bf16, tag="wv", name="wv")
            wp_t = pw.tile([P, 4, C], f32r, tag="wp", name="wp")

            def wcol(dst, src):
                nc.scalar.dma_start(
                    dst, src.rearrange("(i p) c -> p i c", p=P))

            for j in (0,):
                wcol(wqk_t[:, :, DH + P * j : DH + P * (j + 1)],
                     wqk_d[:, DH + P * j : DH + P * (j + 1)])
                wcol(wqk_t[:, :, P * j : P * (j + 1)],
                     wqk_d[:, P * j : P * (j + 1)])
            for hf in range(2):
                wcol(wv_t[:, :, 256 * hf : 256 * (hf + 1)],
                     wv_d[:, 256 * hf : 256 * (hf + 1)])
            nc.scalar.dma_start(tri_t[:], tri_d[:])
            nc.scalar.dma_start(id_t[:], id_d[:])
            for j in (1, 2, 3):
                wcol(wqk_t[:, :, DH + P * j : DH + P * (j + 1)],
                     wqk_d[:, DH + P * j : DH + P * (j + 1)])
                wcol(wqk_t[:, :, P * j : P * (j + 1)],
                     wqk_d[:, P * j : P * (j + 1)])
            for hf in range(2):
                wcol(wp_t[:, :, 512 * hf : 512 * (hf + 1)],
                     wp_d[:, 512 * hf : 512 * (hf + 1)])
            nc.vector.memset(ones_c[:], 1.0)
            nc.vector.tensor_copy(ones_f[64:65, :], id_t[64:65, P : P + 64])

            # ---- persistent stores ----
            # K^T / Q^T bf16 per (feat-pair hp, q-window m): [128 feat, 512]
            k_sb = [
                [pk.tile([P, QC], bf16, tag=f"k{j}_{m}", name=f"k{j}_{m}")
                 for m in range(NQC)]
                for j in range(4)
            ]
            q_sb = [
                [pq.tile([P, QC], bf16, tag=f"q{j}_{m}", name=f"q{j}_{m}")
                 for m in range(NQC)]
                for j in range(4)
            ]
            # V' bf16 per k-block: [128 k, 8 heads, 65] (col 64 = ones)
            vp = [pv.tile([P, 8, 65], bf16, tag=f"vp{kb}", name=f"vp{kb}")
                  for kb in range(NKB)]
            # O^T f32r per (feat-pair hp, q-chunk n): [128 feat, 512 q]
            o_sb = [
                [posb.tile([P, QC], f32r, tag=f"o{i}_{m}", name=f"o{i}_{m}")
                 for m in range(NQC)]
                for i in range(4)
            ]

            # ---- filler units (software pipelining) ----
            def u_qk(kind, j, m):
                def emit():
                    pp = psX.tile([P, QC], f32, tag="fl",
                                  name=f"{kind}ps{j}_{m}")
                    off = 0 if kind == "q" else DH
                    for i in range(NCC):
                        nc.tensor.matmul(
                            pp[:],
                            wqk_t[:, i, off + P * j : off + P * (j + 1)],
                            x_t[i][:, QC * m : QC * (m + 1)],
                            start=(i == 0),
                            stop=(i == NCC - 1),
                        )
                    dst = q_sb if kind == "q" else k_sb
                    nc.vector.tensor_copy(dst[j][m][:], pp[:])
                return emit

            def u_v(kb):
                def emit():
                    pp = psX.tile([P, QC], f32, tag="fl", name=f"vps{kb}")
                    for i in range(NCC):
                        nc.tensor.matmul(
                            pp[:],
                            x_t[i][:, P * kb : P * (kb + 1)],
                            wv_t[:, i, :],
                            start=(i == 0),
                            stop=(i == NCC - 1),
                        )
                    nc.vector.tensor_copy(
                        vp[kb][:, :, 0:64],
                        pp[:].rearrange("p (h d) -> p h d", d=64),
                    )
                    nc.vector.tensor_copy(vp[kb][:, :, 64:65], ones_c[:])
                return emit

            def u_proj(n, j2):
                def emit():
                    pp = psX.tile([P, QC], f32, tag="fl", name=f"pj{j2}_{n}")
                    for i2 in range(4):
                        nc.tensor.matmul(
                            pp[:],
                            wp_t[:, i2, P * j2 : P * (j2 + 1)],
                            o_sb[i2][n][:],
                            start=(i2 == 0),
                            stop=(i2 == 3),
                        )
                    oo = psb.tile([P, QC], f32, tag="oo", name=f"oo{j2}_{n}")
                    nc.vector.tensor_copy(oo[:], pp[:])
                    q = nc.sync if j2 % 2 == 0 else nc.scalar
                    q.dma_start(
                        out_d[P * j2 : P * (j2 + 1), QC * n : QC * (n + 1)],
                        oo[:],
                    )
                return emit

            def qkv_units(m):
                us = [u_qk("k", 0, m), u_qk("q", 0, m)]
                us += [u_v(4 * m + t) for t in range(4)]
                for j in range(1, 4):
                    us += [u_qk("k", j, m), u_qk("q", j, m)]
                return us

            # ---- prologue: QKV for chunk 0 ----
            for u in qkv_units(0):
                u()

            # ---- attention chunks with wedged fillers ----
            FQ = deque()
            pace = {"seen": 0, "emitted": 0, "points": 1, "units": 0}

            def phase(units, points):
                FQ.extend(units)
                pace["seen"] = 0
                pace["emitted"] = 0
                pace["points"] = max(points, 1)
                pace["units"] = len(FQ)

            def wedge():
                pace["seen"] += 1
                while (FQ and pace["emitted"] * pace["points"]
                       < pace["seen"] * pace["units"]):
                    FQ.popleft()()
                    pace["emitted"] += 1

            # rep/recip/mult for head h are deferred into head h+1's stream so
            # the PE never waits on the DVE sums-copy latency.
            pend = []

            def flush_norm():
                while pend:
                    pend.pop(0)()

            for n in range(NQC):
                nkb = 4 * (n + 1)
                ngrp = nkb // GS
                if n < 3:
                    phase(qkv_units(n + 1), 8 * (ngrp + 2))
                else:
                    phase([u_proj(np_, j2) for np_ in range(3)
                           for j2 in range(8)], 8 * (ngrp + 2))

                for h in range(8):
                    hp, par = divmod(h, 2)
                    r0 = 64 * par
                    po = psX.tile([P, QC], f32, tag="po", name=f"po{h}_{n}")
                    for gi in range(ngrp):
                        grp = [GS * gi, GS * gi + 1]
                        sg = psS.tile([P, GS * QC], f32, tag="sg",
                                      name=f"sg{h}_{n}_{gi}")
                        for t_i, kb in enumerate(grp):
                            lhs_k = k_sb[hp][kb // 4][
                                r0 : r0 + 64,
                                P * (kb % 4) : P * (kb % 4 + 1),
                            ]
                            if kb >= 4 * n:
                                tt = kb - 4 * n
                                c0 = P * tt
                                nc.tensor.matmul(
                                    sg[:, QC * t_i + c0 : QC * t_i + c0 + P],
                                    id_t[:, 0:P], tri_t[:],
                                    start=True, stop=True,
                                )
                                nc.tensor.matmul(
                                    sg[:, QC * t_i + c0 : QC * (t_i + 1)],
                                    lhs_k,
                                    q_sb[hp][n][r0 : r0 + 64, c0:QC],
                                    start=False, stop=True,
                                )
                            else:
                                nc.tensor.matmul(
                                    sg[:, QC * t_i : QC * (t_i + 1)],
                                    lhs_k,
                                    q_sb[hp][n][r0 : r0 + 64, :],
                                    start=True, stop=True,
                                )
                        at = pa.tile([P, GS * QC], bf16, tag="at",
                                     name=f"at{h}_{n}_{gi}")
                        nc.scalar.activation(at[:], sg[:], EXP, scale=0.125)
                        wedge()
                        for t_i, kb in enumerate(grp):
                            c0 = P * (kb - 4 * n) if kb >= 4 * n else 0
                            nc.tensor.matmul(
                                po[0:65, c0:QC],
                                vp[kb][:, h, :],
                                at[:, QC * t_i + c0 : QC * (t_i + 1)],
                                start=(kb == 0),
                                stop=(kb >= 4 * n),
                            )
                        if gi == 0:
                            flush_norm()
                    # ---- normalize: sums -> replicate -> recip -> mult ----
                    # sums copy issues now (DVE); the PE/DVE tail is deferred
                    # into the next head's stream.
                    sums = psb.tile([P, QC], f32r, tag="sm", name=f"sm{h}_{n}")
                    nc.vector.tensor_copy(sums[64:65, :], po[64:65, :])
                    wedge()

                    def norm_tail(h=h, n=n, hp=hp, par=par, po=po, sums=sums):
                        rep = psX.tile([P, QC], f32, tag="fl",
                                       name=f"rep{h}_{n}")
                        rr = psb.tile([P, QC], f32, tag="rr", name=f"rr{h}_{n}")
                        nc.tensor.matmul(rep[0:64, :], ones_f[64:65, :],
                                         sums[64:65, :], start=True, stop=True)
                        nc.vector.reciprocal_approx_fast(rr[0:64, :],
                                                         rep[0:64, :])
                        if par == 0:
                            nc.vector.tensor_tensor(o_sb[hp][n][0:64, :],
                                                    po[0:64, :], rr[0:64, :],
                                                    MULT)
                        else:
                            onorm = psb.tile([P, QC], f32r, tag="on",
                                             name=f"on{h}_{n}")
                            nc.vector.tensor_tensor(onorm[0:64, :],
                                                    po[0:64, :], rr[0:64, :],
                                                    MULT)
                            nc.scalar.dma_start(o_sb[hp][n][64:128, :],
                                                onorm[0:64, :])

                    pend.append(norm_tail)
                    wedge()

            # drain any leftover fillers, then final projection
            flush_norm()
            while FQ:
                FQ.popleft()()
            for j2 in range(8):
                u_proj(3, j2)()

    nc.compile()
    return nc


def _get_nc():
    if "nc" not in _CACHE:
        _CACHE["nc"] = _build()
    return _CACHE["nc"]


def _make_tri():
    # additive causal mask for a 128x128 diagonal block of S^T[k, q]:
    # tri[r, c] = -1e4 where q-col c < k-row r (strictly above diagonal)
    tri = np.zeros((P, P), np.float32)
    for r in range(P):
        tri[r, :r] = -1e4
    return tri.astype(ml_dtypes.bfloat16)


def _in_maps(x, w_qkv, w_proj):
    bf = ml_dtypes.bfloat16
    tri = _make_tri()
    idq = np.concatenate([np.eye(P, dtype=np.float32),
                          np.ones((P, P), np.float32)], axis=1).astype(bf)
    maps = []
    for c in range(8):
        b, hh = divmod(c, 2)
        xT = np.ascontiguousarray(x[b].T).astype(bf)
        qcols = w_qkv[:, DH * hh : DH * hh + DH]
        kcols = w_qkv[:, C + DH * hh : C + DH * hh + DH]
        vcols = w_qkv[:, 2 * C + DH * hh : 2 * C + DH * hh + DH]
        maps.append({
            "xt": xT,
            "wqk": np.concatenate([qcols, kcols], axis=1).astype(bf),
            "wv": np.ascontiguousarray(vcols).astype(bf),
            "wp": np.ascontiguousarray(w_proj[DH * hh : DH * hh + DH, :],
                                       dtype=np.float32),
            "tri": tri,
            "idq": idq,
        })
    return maps


def _run(x, w_qkv, w_proj, trace=False):
    from concourse.bass_utils import run_bass_kernel_spmd

    nc = _get_nc()
    maps = _in_maps(x, w_qkv, w_proj)
    res = run_bass_kernel_spmd(nc, maps, list(range(8)), trace=trace)
    out = np.empty((B, T, C), np.float32)
    for b in range(B):
        out[b] = res.results[2 * b]["outT"].T + res.results[2 * b + 1]["outT"].T
    return out, res


def kernel(**inputs):
    x = np.asarray(inputs["x"], dtype=np.float32)
    w_qkv = np.asarray(inputs["w_qkv"], dtype=np.float32)
    w_proj = np.asarray(inputs["w_proj"], dtype=np.float32)
    out, _ = _run(x, w_qkv, w_proj, trace=False)
    return out


# revision 28
# speedup vs baseline: 1.0814x; 1.0362x over previous
"""Causal self-attention (B=4, T=2048, C=1024, 16 heads) on 8 trn2 NeuronCores.

Sharding: core c handles batch b = c//2 and an 8-head half hh = c%2
(tensor parallel over heads). Each core computes its heads' attention
output projected through its slice of w_proj rows; the host sums the two
partial projections per batch.

Device-side layout (per core):
  - QKV^T orientation: Q^T/K^T [feat, T] come straight out of the QKV
    matmul (lhsT = w chunk, rhs = x^T); V comes out in [T, feat] via the
    swapped orientation (lhsT = x^T chunk, rhs = w_v).
  - Scores are computed transposed, S^T[k, q]; softmax sums ride the
    A@V matmul as a ones-column appended to V (M=65).
  - Causality: strictly-below-diagonal k-blocks are computed full-width;
    the 4 diagonal blocks get a 128-wide additive-mask triangle (PE
    matmul: eye @ tri) and column-restricted score/exp-consume/A@V, so
    no PE rows are spent above the diagonal.
  - exp has no max-subtraction (logits are N(0,1)-ish, safe in fp32),
    computed by ACT with the 1/sqrt(D) fused into its scale imm.
  - A and V' are bf16 so restricted (narrow) A@V matmuls still run at
    1 cycle/row; Q/K bf16; proj weights and O in f32r.
  - The attention stream for chunk n is software-pipelined with filler
    matmuls (QKV for chunk n+1, projection of earlier chunks) so the PE
    array keeps working while ACT computes exp.
"""
import os
import sys
from collections import deque

if "/opt/trn_rl_repo" not in sys.path:
    sys.path.insert(0, "/opt/trn_rl_repo")
# The axon NTFF profiling hook is absent in this container; make sure the
# runner never takes the trace path (BASS_TRACE in the env would crash it).
os.environ.setdefault("BASS_NEVER_TRACE", "1")

import numpy as np
import ml_dtypes

B, T, C = 4, 2048, 1024
NH, D = 16, 64
P = 128
QC = 512           # q-chunk width
NQC = T // QC      # 4
NKB = T // P       # 16 k-blocks
GS = 2             # k-blocks per exp group
DH = 512           # per-core head feature width (8 heads * 64)
NCC = C // P       # 8 contraction chunks for QKV

_CACHE = {}


def _build():
    import concourse.mybir as mybir
    import concourse.tile as tile
    from concourse import bacc

    f32 = mybir.dt.float32
    f32r = mybir.dt.float32r
    bf16 = mybir.dt.bfloat16
    MULT = mybir.AluOpType.mult
    EXP = mybir.ActivationFunctionType.Exp

    nc = bacc.Bacc(None, target_bir_lowering=False, debug=False)

    xt_d = nc.declare_dram_parameter("xt", [C, T], bf16, isOutput=False)
    wqk_d = nc.declare_dram_parameter("wqk", [C, 2 * DH], bf16, isOutput=False)
    wv_d = nc.declare_dram_parameter("wv", [C, DH], bf16, isOutput=False)
    wp_d = nc.declare_dram_parameter("wp", [DH, C], f32r, isOutput=False)
    tri_d = nc.declare_dram_parameter("tri", [P, P], bf16, isOutput=False)
    id_d = nc.declare_dram_parameter("idq", [P, 2 * P], bf16, isOutput=False)
    out_d = nc.declare_dram_parameter("outT", [C, T], f32, isOutput=True)

    with tile.TileContext(nc) as tc:
        with (
            tc.tile_pool(name="pconst", bufs=1) as pconst,
            tc.tile_pool(name="pw", bufs=1) as pw,
            tc.tile_pool(name="px", bufs=1) as px,
            tc.tile_pool(name="pq", bufs=1) as pq,
            tc.tile_pool(name="pk", bufs=1) as pk,
            tc.tile_pool(name="pv", bufs=1) as pv,
            tc.tile_pool(name="pa", bufs=3) as pa,
            tc.tile_pool(name="psb", bufs=2) as psb,
            tc.tile_pool(name="posb", bufs=1) as posb,
            tc.tile_pool(name="psS", bufs=2, space="PSUM") as psS,
            tc.tile_pool(name="psX", bufs=2, space="PSUM") as psX,
        ):
            # ---- constants / weights / full-resident x^T ----
            tri_t = pconst.tile([P, P], bf16, name="tri")
            id_t = pconst.tile([P, 2 * P], bf16, name="idt")
            ones_f = pconst.tile([P, 64], f32r, name="ones_f")
            ones_c = pconst.tile([P, 8, 1], f32, name="ones_c")

            # x^T chunks stream on the SP HWDGE queue; weights go column-block
            # by column-block on the ACT HWDGE queue so the first QKV units
            # unblock after one small DMA each instead of eight full-chunk
            # loads.
            x_t = [px.tile([P, T], bf16, tag=f"x{i}", name=f"x{i}")
                   for i in range(NCC)]
            for q in range(4):
                for i in range(NCC):
                    nc.sync.dma_start(
                        x_t[i][:, QC * q : QC * (q + 1)],
                        xt_d[P * i : P * (i + 1), QC * q : QC * (q + 1)])

            wqk_t = pw.tile([P, NCC, 2 * DH], bf16, tag="wqk", name="wqk")
            wv_t = pw.tile([P, NCC, DH], bf16, tag="wv", name="wv")
            wp_t = pw.tile([P, 4, C], f32r, tag="wp", name="wp")

            def wcol(dst, src):
                nc.scalar.dma_start(
                    dst, src.rearrange("(i p) c -> p i c", p=P))

            for j in (0,):
                wcol(wqk_t[:, :, DH + P * j : DH + P * (j + 1)],
                     wqk_d[:, DH + P * j : DH + P * (j + 1)])
                wcol(wqk_t[:, :, P * j : P * (j + 1)],
                     wqk_d[:, P * j : P * (j + 1)])
            for hf in range(2):
                wcol(wv_t[:, :, 256 * hf : 256 * (hf + 1)],
                     wv_d[:, 256 * hf : 256 * (hf + 1)])
            nc.gpsimd.dma_start(tri_t[:], tri_d[:])
            nc.gpsimd.dma_start(id_t[:], id_d[:])
            for j in (1, 2, 3):
                wcol(wqk_t[:, :, DH + P * j : DH + P * (j + 1)],
                     wqk_d[:, DH + P * j : DH + P * (j + 1)])
                wcol(wqk_t[:, :, P * j : P * (j + 1)],
                     wqk_d[:, P * j : P * (j + 1)])
            for hf in range(2):
                nc.gpsimd.dma_start(
                    wp_t[:, :, 512 * hf : 512 * (hf + 1)],
                    wp_d[:, 512 * hf : 512 * (hf + 1)].rearrange(
                        "(i p) c -> p i c", p=P))
            nc.vector.memset(ones_c[:], 1.0)
            nc.vector.tensor_copy(ones_f[64:65, :], id_t[64:65, P : P + 64])

            # ---- persistent stores ----
            # K^T / Q^T bf16 per (feat-pair hp, q-window m): [128 feat, 512]
            k_sb = [
                [pk.tile([P, QC], bf16, tag=f"k{j}_{m}", name=f"k{j}_{m}")
                 for m in range(NQC)]
                for j in range(4)
            ]
            q_sb = [
                [pq.tile([P, QC], bf16, tag=f"q{j}_{m}", name=f"q{j}_{m}")
                 for m in range(NQC)]
                for j in range(4)
            ]
            # V' bf16 per k-block: [128 k, 8 heads, 65] (col 64 = ones)
            vp = [pv.tile([P, 8, 65], bf16, tag=f"vp{kb}", name=f"vp{kb}")
                  for kb in range(NKB)]
            # O^T f32r per (feat-pair hp, q-chunk n): [128 feat, 512 q]
            o_sb = [
                [posb.tile([P, QC], f32r, tag=f"o{i}_{m}", name=f"o{i}_{m}")
                 for m in range(NQC)]
                for i in range(4)
            ]

            # ---- filler units (software pipelining) ----
            def u_qk(kind, j, m):
                def emit():
                    pp = psX.tile([P, QC], f32, tag="fl",
                                  name=f"{kind}ps{j}_{m}")
                    off = 0 if kind == "q" else DH
                    for i in range(NCC):
                        nc.tensor.matmul(
                            pp[:],
                            wqk_t[:, i, off + P * j : off + P * (j + 1)],
                            x_t[i][:, QC * m : QC * (m + 1)],
                            start=(i == 0),
                            stop=(i == NCC - 1),
                        )
                    dst = q_sb if kind == "q" else k_sb
                    nc.vector.tensor_copy(dst[j][m][:], pp[:])
                return emit

            def u_v(kb):
                def emit():
                    pp = psX.tile([P, QC], f32, tag="fl", name=f"vps{kb}")
                    for i in range(NCC):
                        nc.tensor.matmul(
                            pp[:],
                            x_t[i][:, P * kb : P * (kb + 1)],
                            wv_t[:, i, :],
                            start=(i == 0),
                            stop=(i == NCC - 1),
                        )
                    nc.vector.tensor_copy(
                        vp[kb][:, :, 0:64],
                        pp[:].rearrange("p (h d) -> p h d", d=64),
                    )
                    nc.vector.tensor_copy(vp[kb][:, :, 64:65], ones_c[:])
                return emit

            def u_proj(n, j2, ptag="fl"):
                def emit():
                    pool = psS if ptag == "sg" else psX
                    pp = pool.tile([P, QC], f32, tag=ptag, name=f"pj{j2}_{n}")
                    for i2 in range(4):
                        nc.tensor.matmul(
                            pp[:],
                            wp_t[:, i2, P * j2 : P * (j2 + 1)],
                            o_sb[i2][n][:],
                            start=(i2 == 0),
                            stop=(i2 == 3),
                        )
                    oo = psb.tile([P, QC], f32, tag="oo", name=f"oo{j2}_{n}")
                    nc.vector.tensor_copy(oo[:], pp[:])
                    q = nc.sync if j2 % 2 == 0 else nc.scalar
                    q.dma_start(
                        out_d[P * j2 : P * (j2 + 1), QC * n : QC * (n + 1)],
                        oo[:],
                    )
                return emit

            def qkv_units(m):
                us = [u_qk("k", 0, m), u_qk("q", 0, m)]
                us += [u_v(4 * m + t) for t in range(4)]
                for j in range(1, 4):
                    us += [u_qk("k", j, m), u_qk("q", j, m)]
                return us

            # ---- prologue: the minimal QKV prefix head 0/1 of chunk 0
            # needs; the j>0 feature chunks ride the chunk-0 filler queue ----
            prologue = qkv_units(0)
            for u in prologue[:6]:
                u()
            rest0 = prologue[6:]

            # ---- attention chunks with wedged fillers ----
            FQ = deque()
            pace = {"seen": 0, "emitted": 0, "points": 1, "units": 0}

            def phase(units, points):
                FQ.extend(units)
                pace["seen"] = 0
                pace["emitted"] = 0
                pace["points"] = max(points, 1)
                pace["units"] = len(FQ)

            def wedge():
                pace["seen"] += 1
                while (FQ and pace["emitted"] * pace["points"]
                       < pace["seen"] * pace["units"]):
                    FQ.popleft()()
                    pace["emitted"] += 1

            # rep/recip/mult for head h are deferred into head h+1's stream so
            # the PE never waits on the DVE sums-copy latency.
            pend = []

            def flush_norm():
                while pend:
                    pend.pop(0)()

            for n in range(NQC):
                nkb = 4 * (n + 1)
                ngrp = nkb // GS
                if n == 0:
                    phase(rest0 + qkv_units(1), 8 * (ngrp + 2))
                elif n < 3:
                    phase(qkv_units(n + 1), 8 * (ngrp + 2))
                else:
                    phase([u_proj(np_, j2) for np_ in range(3)
                           for j2 in range(8)], 8 * (ngrp + 2))

                # odd heads first: the last head's norm tail then has no
                # partition-shift DMA on the end-of-phase critical path
                for h in (1, 3, 5, 7, 0, 2, 4, 6):
                    hp, par = divmod(h, 2)
                    r0 = 64 * par
                    po = psX.tile([P, QC], f32, tag="po", name=f"po{h}_{n}")
                    for gi in range(ngrp):
                        grp = [GS * gi, GS * gi + 1]
                        sg = psS.tile([P, GS * QC], f32, tag="sg",
                                      name=f"sg{h}_{n}_{gi}")
                        for t_i, kb in enumerate(grp):
                            lhs_k = k_sb[hp][kb // 4][
                                r0 : r0 + 64,
                                P * (kb % 4) : P * (kb % 4 + 1),
                            ]
                            if kb >= 4 * n:
                                tt = kb - 4 * n
                                c0 = P * tt
                                nc.tensor.matmul(
                                    sg[:, QC * t_i + c0 : QC * t_i + c0 + P],
                                    id_t[:, 0:P], tri_t[:],
                                    start=True, stop=True,
                                )
                                nc.tensor.matmul(
                                    sg[:, QC * t_i + c0 : QC * (t_i + 1)],
                                    lhs_k,
                                    q_sb[hp][n][r0 : r0 + 64, c0:QC],
                                    start=False, stop=True,
                                )
                            else:
                                nc.tensor.matmul(
                                    sg[:, QC * t_i : QC * (t_i + 1)],
                                    lhs_k,
                                    q_sb[hp][n][r0 : r0 + 64, :],
                                    start=True, stop=True,
                                )
                        at = pa.tile([P, GS * QC], bf16, tag="at",
                                     name=f"at{h}_{n}_{gi}")
                        nc.scalar.activation(at[:], sg[:], EXP, scale=0.125)
                        wedge()
                        for t_i, kb in enumerate(grp):
                            c0 = P * (kb - 4 * n) if kb >= 4 * n else 0
                            nc.tensor.matmul(
                                po[0:65, c0:QC],
                                vp[kb][:, h, :],
                                at[:, QC * t_i + c0 : QC * (t_i + 1)],
                                start=(kb == 0),
                                stop=(kb >= 4 * n),
                            )
                        if gi == 0:
                            flush_norm()
                    # ---- normalize: sums -> replicate -> recip -> mult ----
                    # sums copy issues now (DVE); the PE/DVE tail is deferred
                    # into the next head's stream.
                    sums = psb.tile([P, QC], f32r, tag="sm", name=f"sm{h}_{n}")
                    nc.vector.tensor_copy(sums[64:65, :], po[64:65, :])
                    wedge()

                    def norm_tail(h=h, n=n, hp=hp, par=par, po=po, sums=sums):
                        rep = psX.tile([P, QC], f32, tag="fl",
                                       name=f"rep{h}_{n}")
                        rr = psb.tile([P, QC], f32, tag="rr", name=f"rr{h}_{n}")
                        nc.tensor.matmul(rep[0:64, :], ones_f[64:65, :],
                                         sums[64:65, :], start=True, stop=True)
                        nc.vector.reciprocal_approx_fast(rr[0:64, :],
                                                         rep[0:64, :])
                        if par == 0:
                            nc.vector.tensor_tensor(o_sb[hp][n][0:64, :],
                                                    po[0:64, :], rr[0:64, :],
                                                    MULT)
                        else:
                            onorm = psb.tile([P, QC], f32r, tag="on",
                                             name=f"on{h}_{n}")
                            nc.vector.tensor_tensor(onorm[0:64, :],
                                                    po[0:64, :], rr[0:64, :],
                                                    MULT)
                            nc.scalar.dma_start(o_sb[hp][n][64:128, :],
                                                onorm[0:64, :])

                    pend.append(norm_tail)
                    wedge()

            # drain any leftover fillers, then final projection (alternating
            # PSUM tags: the attention rings are idle now, so 4 units pipeline)
            flush_norm()
            while FQ:
                FQ.popleft()()
            for j2 in range(8):
                u_proj(3, j2, ptag="fl" if j2 % 2 == 0 else "sg")()

    nc.compile()
    return nc


def _get_nc():
    if "nc" not in _CACHE:
        _CACHE["nc"] = _build()
    return _CACHE["nc"]


def _make_tri():
    # additive causal mask for a 128x128 diagonal block of S^T[k, q]:
    # tri[r, c] = -1e4 where q-col c < k-row r (strictly above diagonal)
    tri = np.zeros((P, P), np.float32)
    for r in range(P):
        tri[r, :r] = -1e4
    return tri.astype(ml_dtypes.bfloat16)


def _in_maps(x, w_qkv, w_proj):
    bf = ml_dtypes.bfloat16
    tri = _make_tri()
    idq = np.concatenate([np.eye(P, dtype=np.float32),
                          np.ones((P, P), np.float32)], axis=1).astype(bf)
    maps = []
    for c in range(8):
        b, hh = divmod(c, 2)
        xT = np.ascontiguousarray(x[b].T).astype(bf)
        qcols = w_qkv[:, DH * hh : DH * hh + DH]
        kcols = w_qkv[:, C + DH * hh : C + DH * hh + DH]
        vcols = w_qkv[:, 2 * C + DH * hh : 2 * C + DH * hh + DH]
        maps.append({
            "xt": xT,
            "wqk": np.concatenate([qcols, kcols], axis=1).astype(bf),
            "wv": np.ascontiguousarray(vcols).astype(bf),
            "wp": np.ascontiguousarray(w_proj[DH * hh : DH * hh + DH, :],
                                       dtype=np.float32),
            "tri": tri,
            "idq": idq,
        })
    return maps


def _run(x, w_qkv, w_proj, trace=False):
    from concourse.bass_utils import run_bass_kernel_spmd

    nc = _get_nc()
    maps = _in_maps(x, w_qkv, w_proj)
    res = run_bass_kernel_spmd(nc, maps, list(range(8)), trace=trace)
    out = np.empty((B, T, C), np.float32)
    for b in range(B):
        out[b] = res.results[2 * b]["outT"].T + res.results[2 * b + 1]["outT"].T
    return out, res


def kernel(**inputs):
    x = np.asarray(inputs["x"], dtype=np.float32)
    w_qkv = np.asarray(inputs["w_qkv"], dtype=np.float32)
    w_proj = np.asarray(inputs["w_proj"], dtype=np.float32)
    out, _ = _run(x, w_qkv, w_proj, trace=False)
    return out


# revision 74
# speedup vs baseline: 1.2242x; 1.1321x over previous
"""Causal self-attention (B=4, T=2048, C=1024, 16 heads) on 8 trn2 NeuronCores.

Sharding: core c handles batch b = c//2 and an 8-head half hh = c%2
(tensor parallel over heads). Each core computes its heads' attention
output projected through its slice of w_proj rows; the host sums the two
partial projections per batch.

Device-side layout (per core):
  - QKV^T orientation: Q^T/K^T [feat, T] come straight out of the QKV
    matmul (lhsT = w chunk, rhs = x^T); V comes out in [T, feat] via the
    swapped orientation (lhsT = x^T chunk, rhs = w_v).
  - Scores are computed transposed, S^T[k, q]; softmax sums ride the
    A@V matmul as a ones-column appended to V (M=65).
  - Causality: strictly-below-diagonal k-blocks are computed full-width;
    the 4 diagonal blocks get a 128-wide additive-mask triangle (PE
    matmul: eye @ tri) and column-restricted score/exp-consume/A@V, so
    no PE rows are spent above the diagonal.
  - exp has no max-subtraction (logits are N(0,1)-ish, safe in fp32),
    computed by ACT with the 1/sqrt(D) fused into its scale imm.
  - A and V' are bf16 so restricted (narrow) A@V matmuls still run at
    1 cycle/row; Q/K bf16; proj weights and O in f32r.
  - The attention stream for chunk n is software-pipelined with filler
    matmuls (QKV for chunk n+1, projection of earlier chunks) so the PE
    array keeps working while ACT computes exp.
"""
import os
import sys
from collections import deque

if "/opt/trn_rl_repo" not in sys.path:
    sys.path.insert(0, "/opt/trn_rl_repo")
# The axon NTFF profiling hook is absent in this container; make sure the
# runner never takes the trace path (BASS_TRACE in the env would crash it).
os.environ.setdefault("BASS_NEVER_TRACE", "1")

import numpy as np
import ml_dtypes

B, T, C = 4, 2048, 1024
NH, D = 16, 64
P = 128
QC = 512           # q-chunk width
NQC = T // QC      # 4
NKB = T // P       # 16 k-blocks
GS = 2             # k-blocks per exp group
DH = 512           # per-core head feature width (8 heads * 64)
NCC = C // P       # 8 contraction chunks for QKV

_CACHE = {}


def _build():
    import concourse.mybir as mybir
    import concourse.tile as tile
    from concourse import bacc

    f32 = mybir.dt.float32
    f32r = mybir.dt.float32r
    bf16 = mybir.dt.bfloat16
    MULT = mybir.AluOpType.mult
    EXP = mybir.ActivationFunctionType.Exp

    nc = bacc.Bacc(None, target_bir_lowering=False, debug=False)

    xt_d = nc.declare_dram_parameter("xt", [C, T], bf16, isOutput=False)
    wqk_d = nc.declare_dram_parameter("wqk", [C, 2 * DH], bf16, isOutput=False)
    wv_d = nc.declare_dram_parameter("wv", [C, DH], bf16, isOutput=False)
    wp_d = nc.declare_dram_parameter("wp", [DH, C], f32r, isOutput=False)
    tri_d = nc.declare_dram_parameter("tri", [P, P], bf16, isOutput=False)
    id_d = nc.declare_dram_parameter("idq", [P, 2 * P], bf16, isOutput=False)
    out_d = nc.declare_dram_parameter("outT", [C, T], f32, isOutput=True)

    with tile.TileContext(nc) as tc:
        with (
            tc.tile_pool(name="pconst", bufs=1) as pconst,
            tc.tile_pool(name="pw", bufs=1) as pw,
            tc.tile_pool(name="px", bufs=1) as px,
            tc.tile_pool(name="pq", bufs=1) as pq,
            tc.tile_pool(name="pk", bufs=1) as pk,
            tc.tile_pool(name="pv", bufs=1) as pv,
            tc.tile_pool(name="pa", bufs=4) as pa,
            tc.tile_pool(name="psb", bufs=2) as psb,
            tc.tile_pool(name="posb", bufs=1) as posb,
            tc.tile_pool(name="psS", bufs=2, space="PSUM") as psS,
            tc.tile_pool(name="psX", bufs=2, space="PSUM") as psX,
        ):
            # ---- constants / weights / full-resident x^T ----
            tri_t = pconst.tile([P, P], bf16, name="tri")
            id_t = pconst.tile([P, 2 * P], bf16, name="idt")
            ones_c = pconst.tile([P, 8, 1], f32, name="ones_c")

            # x^T chunks stream on the SP HWDGE queue; weights go column-block
            # by column-block on the ACT HWDGE queue so the first QKV units
            # unblock after one small DMA each instead of eight full-chunk
            # loads.
            # All input loads ride ONE HWDGE queue in exact consumption
            # order: the DMA transfer device is serial (FIFO by ready time),
            # so queue order == transfer priority.
            x_t = [px.tile([P, T], bf16, tag=f"x{i}", name=f"x{i}")
                   for i in range(NCC)]
            wqk_t = pw.tile([P, NCC, 2 * DH], bf16, tag="wqk", name="wqk")
            wv_t = pw.tile([P, NCC, DH], bf16, tag="wv", name="wv")
            wp_t = pw.tile([P, 4, C], f32r, tag="wp", name="wp")

            def wcol(dst, src):
                nc.sync.dma_start(
                    dst, src.rearrange("(i p) c -> p i c", p=P))

            def ldx(q):
                for i in range(NCC):
                    nc.sync.dma_start(
                        x_t[i][:, QC * q : QC * (q + 1)],
                        xt_d[P * i : P * (i + 1), QC * q : QC * (q + 1)])

            def ldqk(j, off, ih=None):
                if ih is None:
                    wcol(wqk_t[:, :, off + P * j : off + P * (j + 1)],
                         wqk_d[:, off + P * j : off + P * (j + 1)])
                else:
                    wcol(wqk_t[:, 4 * ih : 4 * ih + 4,
                               off + P * j : off + P * (j + 1)],
                         wqk_d[512 * ih : 512 * ih + 512,
                               off + P * j : off + P * (j + 1)])

            def ldx1(q, i):
                nc.sync.dma_start(
                    x_t[i][:, QC * q : QC * (q + 1)],
                    xt_d[P * i : P * (i + 1), QC * q : QC * (q + 1)])

            # weight half-columns woven between the x chunks they pace
            ldqk(0, DH, 0)
            ldx1(0, 0)
            ldx1(0, 1)
            ldqk(0, DH, 1)
            ldx1(0, 2)
            ldx1(0, 3)
            ldqk(0, 0, 0)
            ldx1(0, 4)
            ldx1(0, 5)
            ldqk(0, 0, 1)
            ldx1(0, 6)
            ldx1(0, 7)
            nc.sync.dma_start(tri_t[:], tri_d[:])
            nc.sync.dma_start(id_t[:], id_d[:])
            for ih in range(2):
                wcol(wv_t[:, 4 * ih : 4 * ih + 4, :],
                     wv_d[512 * ih : 512 * ih + 512, :])
            for off in (DH, 0):
                ldqk(1, off)
            ldx(1)
            for off in (DH, 0):
                ldqk(2, off)
            ldx(2)
            for off in (DH, 0):
                ldqk(3, off)
            ldx(3)
            for hf in range(2):
                wcol(wp_t[:, :, 512 * hf : 512 * (hf + 1)],
                     wp_d[:, 512 * hf : 512 * (hf + 1)])
            nc.vector.memset(ones_c[:], 1.0)

            # ---- persistent stores ----
            # K^T / Q^T bf16 per (feat-pair hp, q-window m): [128 feat, 512]
            k_sb = [
                [pk.tile([P, QC], bf16, tag=f"k{j}_{m}", name=f"k{j}_{m}")
                 for m in range(NQC)]
                for j in range(4)
            ]
            q_sb = [
                [pq.tile([P, QC], bf16, tag=f"q{j}_{m}", name=f"q{j}_{m}")
                 for m in range(NQC)]
                for j in range(4)
            ]
            # V' bf16 per k-block: [128 k, 8 heads, 65] (col 64 = ones)
            vp = [pv.tile([P, 8, 65], bf16, tag=f"vp{kb}", name=f"vp{kb}")
                  for kb in range(NKB)]
            # O^T f32r per (feat-pair hp, q-chunk n): [128 feat, 512 q]
            o_sb = [
                [posb.tile([P, QC], f32r, tag=f"o{i}_{m}", name=f"o{i}_{m}")
                 for m in range(NQC)]
                for i in range(4)
            ]

            # ---- filler units (software pipelining) ----
            # each unit is split into two half-thunks (4 matmuls each) so the
            # wedge points get finer-grained PE work
            def u_qk(kind, j, m):
                box = {}
                off = 0 if kind == "q" else DH

                def mm(pp, i):
                    nc.tensor.matmul(
                        pp[:],
                        wqk_t[:, i, off + P * j : off + P * (j + 1)],
                        x_t[i][:, QC * m : QC * (m + 1)],
                        start=(i == 0),
                        stop=(i == NCC - 1),
                    )

                def emit_a():
                    box["pp"] = psX.tile([P, QC], f32, tag="fl",
                                         name=f"{kind}ps{j}_{m}")
                    for i in range(4):
                        mm(box["pp"], i)

                def emit_b():
                    for i in range(4, NCC):
                        mm(box["pp"], i)
                    dst = q_sb if kind == "q" else k_sb
                    nc.vector.tensor_copy(dst[j][m][:], box["pp"][:])
                return emit_a, emit_b

            def u_v(kb):
                box = {}

                def mm(pp, i):
                    nc.tensor.matmul(
                        pp[:],
                        x_t[i][:, P * kb : P * (kb + 1)],
                        wv_t[:, i, :],
                        start=(i == 0),
                        stop=(i == NCC - 1),
                    )

                def emit_a():
                    box["pp"] = psX.tile([P, QC], f32, tag="fl",
                                         name=f"vps{kb}")
                    for i in range(4):
                        mm(box["pp"], i)

                def emit_b():
                    for i in range(4, NCC):
                        mm(box["pp"], i)
                    nc.vector.tensor_copy(
                        vp[kb][:, :, 0:64],
                        box["pp"][:].rearrange("p (h d) -> p h d", d=64),
                    )
                    nc.vector.tensor_copy(vp[kb][:, :, 64:65], ones_c[:])
                return emit_a, emit_b

            def pj_mm(pp, n, j2, i2):
                nc.tensor.matmul(
                    pp[:],
                    wp_t[:, i2, P * j2 : P * (j2 + 1)],
                    o_sb[i2][n][:],
                    start=(i2 == 0),
                    stop=(i2 == 3),
                )

            def pj_out(pp, n, j2, act_copy):
                oo = psb.tile([P, QC], f32, tag="oo", bufs=8,
                              name=f"oo{j2}_{n}")
                if act_copy:
                    nc.scalar.copy(oo[:], pp[:])
                else:
                    nc.vector.tensor_copy(oo[:], pp[:])
                q = nc.sync if j2 % 2 == 0 else nc.scalar
                q.dma_start(
                    out_d[P * j2 : P * (j2 + 1), QC * n : QC * (n + 1)],
                    oo[:],
                )

            def u_proj(n, j2, ptag="fl", act_copy=False):
                def emit():
                    pool = psS if ptag == "sg" else psX
                    pp = pool.tile([P, QC], f32, tag=ptag, name=f"pj{j2}_{n}")
                    for i2 in range(4):
                        pj_mm(pp, n, j2, i2)
                    pj_out(pp, n, j2, act_copy)
                return emit

            pj3_pp = {}

            def u_proj3_partial(j2):
                def emit():
                    pp = psX.tile([P, QC], f32, tag="fl", name=f"pj{j2}_3")
                    pj3_pp[j2] = pp
                    for i2 in range(3):
                        pj_mm(pp, 3, j2, i2)
                return emit

            def qkv_units(m):
                us = [*u_qk("k", 0, m), *u_qk("q", 0, m)]
                for t in range(4):
                    us += [*u_v(4 * m + t)]
                for j in range(1, 4):
                    us += [*u_qk("k", j, m), *u_qk("q", j, m)]
                return us

            # ---- attention chunks with wedged fillers ----
            # FQ entries are (absolute-deadline-wedge-index, emit_fn).
            # Bresenham pacing spreads units across the phase; the deadline
            # forces a unit out before the attention instruction that reads
            # its output is emitted (program order defines data flow).
            FQ = deque()
            pace = {"W": 0, "seen": 0, "emitted": 0, "points": 1, "units": 0}

            def phase(units, points):
                FQ.extend(units)
                pace["seen"] = 0
                pace["emitted"] = 0
                pace["points"] = max(points, 1)
                pace["units"] = len(FQ)

            def wedge():
                pace["W"] += 1
                pace["seen"] += 1
                while FQ and (
                    pace["emitted"] * pace["points"]
                    < pace["seen"] * pace["units"]
                    or FQ[0][0] <= pace["W"]
                ):
                    FQ.popleft()[1]()
                    pace["emitted"] += 1

            def qkv_deadlines(m, base):
                # phase m: 8 heads x (ngrp + 3) wedge points; head-pair hp's
                # first consumer (odd head 2hp+1) sits at position hp
                ngrp_m = 2 * (m + 1)
                span = ngrp_m + 3
                dk = lambda j: base + j * span
                dv = lambda kb: base + 1 + ((kb % 4) // 2)
                us = qkv_units(m)
                dl = [dk(0), dk(0), dv(4 * m), dv(4 * m + 1), dv(4 * m + 2),
                      dv(4 * m + 3), dk(1), dk(1), dk(2), dk(2), dk(3), dk(3)]
                dl2 = [d for d in dl for _ in range(2)]
                return list(zip(dl2, us))

            # ---- prologue: the minimal QKV prefix head 0/1 of chunk 0
            # needs; V and the j>0 feature chunks ride the chunk-0 filler
            # queue so their wv/x waits never block the attention start ----
            prologue = qkv_deadlines(0, 0)
            for _, u in prologue[:4]:
                u()
            rest0 = prologue[4:]

            for n in range(NQC):
                nkb = 4 * (n + 1)
                ngrp = nkb // GS
                points = 8 * (ngrp + 3)
                next_base = pace["W"] + points
                if n == 0:
                    phase(rest0 + qkv_deadlines(1, next_base), points)
                elif n < 3:
                    phase(qkv_deadlines(n + 1, next_base), points)
                else:
                    phase([(1 << 30, u_proj(np_, j2)) for np_ in range(3)
                           for j2 in range(8)]
                          + [(1 << 30, u_proj3_partial(j2)) for j2 in (6, 7)],
                          points)

                # pair-major order so o_sb feature chunks complete early for
                # the projection
                for h in (1, 0, 3, 2, 5, 4, 7, 6):
                    wedge()
                    hp, par = divmod(h, 2)
                    r0 = 64 * par
                    po = psX.tile([P, QC], f32, tag="po", name=f"po{h}_{n}")
                    def emit_av(grp, at):
                        for t_i, kb in enumerate(grp):
                            c0 = P * (kb - 4 * n) if kb >= 4 * n else 0
                            nc.tensor.matmul(
                                po[0:65, c0:QC],
                                vp[kb][:, h, :],
                                at[:, QC * t_i + c0 : QC * (t_i + 1)],
                                start=(kb == 0),
                                stop=(kb >= 4 * n),
                            )

                    # A@V is skewed one group behind the scores so the exp
                    # latency is covered by the next group's score matmuls
                    pend_av = None
                    for gi in range(ngrp):
                        grp = [GS * gi, GS * gi + 1]
                        sg = psS.tile([P, GS * QC], f32, tag="sg",
                                      name=f"sg{h}_{n}_{gi}")
                        for t_i, kb in enumerate(grp):
                            lhs_k = k_sb[hp][kb // 4][
                                r0 : r0 + 64,
                                P * (kb % 4) : P * (kb % 4 + 1),
                            ]
                            if kb >= 4 * n:
                                tt = kb - 4 * n
                                c0 = P * tt
                                nc.tensor.matmul(
                                    sg[:, QC * t_i + c0 : QC * t_i + c0 + P],
                                    id_t[:, 0:P], tri_t[:],
                                    start=True, stop=True,
                                )
                                nc.tensor.matmul(
                                    sg[:, QC * t_i + c0 : QC * (t_i + 1)],
                                    lhs_k,
                                    q_sb[hp][n][r0 : r0 + 64, c0:QC],
                                    start=False, stop=True,
                                )
                            else:
                                nc.tensor.matmul(
                                    sg[:, QC * t_i : QC * (t_i + 1)],
                                    lhs_k,
                                    q_sb[hp][n][r0 : r0 + 64, :],
                                    start=True, stop=True,
                                )
                        at = pa.tile([P, GS * QC], bf16, tag="at",
                                     name=f"at{h}_{n}_{gi}")
                        if gi == ngrp - 1:
                            # last group = diag blocks tt2/tt3: exp only the
                            # causally-needed column ranges
                            nc.scalar.activation(at[:, 256:512],
                                                 sg[:, 256:512], EXP,
                                                 scale=0.125)
                            nc.scalar.activation(at[:, 896:1024],
                                                 sg[:, 896:1024], EXP,
                                                 scale=0.125)
                        else:
                            nc.scalar.activation(at[:], sg[:], EXP,
                                                 scale=0.125)
                        if pend_av is not None:
                            emit_av(*pend_av)
                        wedge()
                        pend_av = (grp, at)
                    emit_av(*pend_av)
                    # ---- normalize: sums -> broadcast -> recip -> mult ----
                    # no PE instruction in this chain: DVE copies the PSUM
                    # sums row, Pool broadcasts, DVE recips and multiplies.
                    sums = psb.tile([P, QC], f32, tag="sm", name=f"sm{h}_{n}")
                    nc.vector.tensor_copy(sums[0:1, :], po[64:65, :])
                    wedge()
                    sb = psb.tile([P, QC], f32, tag="bc", name=f"bc{h}_{n}")
                    rr = psb.tile([P, QC], f32, tag="rr", name=f"rr{h}_{n}")
                    nc.gpsimd.partition_broadcast(sb[0:64, :], sums[0:1, :])
                    nc.vector.reciprocal_approx_fast(rr[0:64, :], sb[0:64, :])
                    dst = (o_sb[hp][n][0:64, :] if par == 0
                           else o_sb[hp][n][64:128, :])
                    nc.vector.tensor_tensor(dst, po[0:64, :], rr[0:64, :],
                                            MULT)
                    wedge()

            # drain any leftover fillers, then final projection (alternating
            # PSUM tags: the attention rings are idle now, so 4 units pipeline)
            while FQ:
                FQ.popleft()[1]()
            # two more partials on the now-free score rings fill the window
            # while the last head's normalization chain completes
            for j2, ptg in ((0, "sg"), (1, "po")):
                pool = psS if ptg == "sg" else psX
                pp = pool.tile([P, QC], f32, tag=ptg, name=f"pj{j2}_3")
                pj3_pp[j2] = pp
                for i2 in range(3):
                    pj_mm(pp, 3, j2, i2)
            for j2 in (6, 7, 0, 1):
                pj_mm(pj3_pp[j2], 3, j2, 3)
                pj_out(pj3_pp[j2], 3, j2, act_copy=(j2 % 2 == 1))
            for j2 in range(2, 6):
                u_proj(3, j2, ptag="fl" if j2 % 2 == 0 else "sg",
                       act_copy=(j2 % 2 == 1))()

    nc.compile()
    return nc


def _get_nc():
    if "nc" not in _CACHE:
        _CACHE["nc"] = _build()
    return _CACHE["nc"]


def _make_tri():
    # additive causal mask for a 128x128 diagonal block of S^T[k, q]:
    # tri[r, c] = -1e4 where q-col c < k-row r (strictly above diagonal)
    tri = np.zeros((P, P), np.float32)
    for r in range(P):
        tri[r, :r] = -1e4
    return tri.astype(ml_dtypes.bfloat16)


def _in_maps(x, w_qkv, w_proj):
    bf = ml_dtypes.bfloat16
    tri = _make_tri()
    idq = np.concatenate([np.eye(P, dtype=np.float32),
                          np.ones((P, P), np.float32)], axis=1).astype(bf)
    maps = []
    for c in range(8):
        b, hh = divmod(c, 2)
        xT = np.ascontiguousarray(x[b].T).astype(bf)
        qcols = w_qkv[:, DH * hh : DH * hh + DH]
        kcols = w_qkv[:, C + DH * hh : C + DH * hh + DH]
        vcols = w_qkv[:, 2 * C + DH * hh : 2 * C + DH * hh + DH]
        maps.append({
            "xt": xT,
            "wqk": np.concatenate([qcols, kcols], axis=1).astype(bf),
            "wv": np.ascontiguousarray(vcols).astype(bf),
            "wp": np.ascontiguousarray(w_proj[DH * hh : DH * hh + DH, :],
                                       dtype=np.float32),
            "tri": tri,
            "idq": idq,
        })
    return maps


def _run(x, w_qkv, w_proj, trace=False):
    from concourse.bass_utils import run_bass_kernel_spmd

    nc = _get_nc()
    maps = _in_maps(x, w_qkv, w_proj)
    res = run_bass_kernel_spmd(nc, maps, list(range(8)), trace=trace)
    out = np.empty((B, T, C), np.float32)
    for b in range(B):
        out[b] = res.results[2 * b]["outT"].T + res.results[2 * b + 1]["outT"].T
    return out, res


def kernel(**inputs):
    x = np.asarray(inputs["x"], dtype=np.float32)
    w_qkv = np.asarray(inputs["w_qkv"], dtype=np.float32)
    w_proj = np.asarray(inputs["w_proj"], dtype=np.float32)
    out, _ = _run(x, w_qkv, w_proj, trace=False)
    return out


# revision 83
# speedup vs baseline: 1.2441x; 1.0162x over previous
"""Causal self-attention (B=4, T=2048, C=1024, 16 heads) on 8 trn2 NeuronCores.

Sharding: core c handles batch b = c//2 and an 8-head half hh = c%2
(tensor parallel over heads). Each core computes its heads' attention
output projected through its slice of w_proj rows; the host sums the two
partial projections per batch.

Device-side layout (per core):
  - QKV^T orientation: Q^T/K^T [feat, T] come straight out of the QKV
    matmul (lhsT = w chunk, rhs = x^T); V comes out in [T, feat] via the
    swapped orientation (lhsT = x^T chunk, rhs = w_v).
  - Scores are computed transposed, S^T[k, q]; softmax sums ride the
    A@V matmul as a ones-column appended to V (M=65).
  - Causality: strictly-below-diagonal k-blocks are computed full-width;
    the 4 diagonal blocks get column-restricted score/exp/A@V plus a
    multiplicative 0/1 triangle applied to the 128-col diagonal band of
    A on the DVE, so no PE rows are spent above the diagonal.
  - exp has no max-subtraction (logits are N(0,1)-ish, safe in fp32),
    computed by ACT with the 1/sqrt(D) fused into its scale imm.
  - A and V' are bf16 so restricted (narrow) A@V matmuls still run at
    1 cycle/row; Q/K bf16; proj weights and O in f32r.
  - Softmax normalization never touches the PE: the DVE copies the PSUM
    sums row cross-partition to partition 0, the Pool engine broadcasts
    it, the DVE recips and multiplies into O^T.
  - The attention stream for chunk n is software-pipelined with filler
    matmuls (QKV for chunk n+1, projection of earlier chunks, deadline-
    paced) so the PE array keeps working while ACT computes exp.
"""
import os
import sys
from collections import deque

if "/opt/trn_rl_repo" not in sys.path:
    sys.path.insert(0, "/opt/trn_rl_repo")
# The axon NTFF profiling hook is absent in this container; make sure the
# runner never takes the trace path (BASS_TRACE in the env would crash it).
os.environ.setdefault("BASS_NEVER_TRACE", "1")

import numpy as np
import ml_dtypes

B, T, C = 4, 2048, 1024
NH, D = 16, 64
P = 128
QC = 512           # q-chunk width
NQC = T // QC      # 4
NKB = T // P       # 16 k-blocks
GS = 2             # k-blocks per exp group
DH = 512           # per-core head feature width (8 heads * 64)
NCC = C // P       # 8 contraction chunks for QKV

_CACHE = {}


def _build():
    import concourse.mybir as mybir
    import concourse.tile as tile
    from concourse import bacc

    f32 = mybir.dt.float32
    f32r = mybir.dt.float32r
    bf16 = mybir.dt.bfloat16
    MULT = mybir.AluOpType.mult
    EXP = mybir.ActivationFunctionType.Exp

    nc = bacc.Bacc(None, target_bir_lowering=False, debug=False)

    xt_d = nc.declare_dram_parameter("xt", [C, T], bf16, isOutput=False)
    wqk_d = nc.declare_dram_parameter("wqk", [C, 2 * DH], bf16, isOutput=False)
    wv_d = nc.declare_dram_parameter("wv", [C, DH], bf16, isOutput=False)
    wp_d = nc.declare_dram_parameter("wp", [DH, C], f32r, isOutput=False)
    tri_d = nc.declare_dram_parameter("tri", [P, P], bf16, isOutput=False)
    out_d = nc.declare_dram_parameter("outT", [C, T], f32, isOutput=True)

    with tile.TileContext(nc) as tc:
        with (
            tc.tile_pool(name="pconst", bufs=1) as pconst,
            tc.tile_pool(name="pw", bufs=1) as pw,
            tc.tile_pool(name="px", bufs=1) as px,
            tc.tile_pool(name="pq", bufs=1) as pq,
            tc.tile_pool(name="pk", bufs=1) as pk,
            tc.tile_pool(name="pv", bufs=1) as pv,
            tc.tile_pool(name="pa", bufs=4) as pa,
            tc.tile_pool(name="psb", bufs=2) as psb,
            tc.tile_pool(name="posb", bufs=1) as posb,
            tc.tile_pool(name="psS", bufs=2, space="PSUM") as psS,
            tc.tile_pool(name="psX", bufs=2, space="PSUM") as psX,
        ):
            # ---- constants / weights / full-resident x^T ----
            tri_t = pconst.tile([P, P], bf16, name="tri")
            ones_c = pconst.tile([P, 8, 1], f32, name="ones_c")

            # All input loads ride ONE HWDGE queue in exact consumption
            # order: the DMA transfer device is serial (FIFO by ready time),
            # so queue order == transfer priority.
            x_t = [px.tile([P, T], bf16, tag=f"x{i}", name=f"x{i}")
                   for i in range(NCC)]
            wqk_t = pw.tile([P, NCC, 2 * DH], bf16, tag="wqk", name="wqk")
            wv_t = pw.tile([P, NCC, DH], bf16, tag="wv", name="wv")
            wp_t = pw.tile([P, 4, C], f32r, tag="wp", name="wp")

            def wcol(dst, src):
                nc.sync.dma_start(
                    dst, src.rearrange("(i p) c -> p i c", p=P))

            def ldx(q):
                for i in range(NCC):
                    nc.sync.dma_start(
                        x_t[i][:, QC * q : QC * (q + 1)],
                        xt_d[P * i : P * (i + 1), QC * q : QC * (q + 1)])

            def ldqk(j, off, ih=None):
                if ih is None:
                    wcol(wqk_t[:, :, off + P * j : off + P * (j + 1)],
                         wqk_d[:, off + P * j : off + P * (j + 1)])
                else:
                    wcol(wqk_t[:, 4 * ih : 4 * ih + 4,
                               off + P * j : off + P * (j + 1)],
                         wqk_d[512 * ih : 512 * ih + 512,
                               off + P * j : off + P * (j + 1)])

            def ldx1(q, i):
                nc.sync.dma_start(
                    x_t[i][:, QC * q : QC * (q + 1)],
                    xt_d[P * i : P * (i + 1), QC * q : QC * (q + 1)])

            # weight half-columns woven between the x chunks they pace
            ldqk(0, DH, 0)
            ldx1(0, 0)
            ldx1(0, 1)
            ldqk(0, DH, 1)
            ldx1(0, 2)
            ldx1(0, 3)
            ldqk(0, 0, 0)
            ldx1(0, 4)
            ldx1(0, 5)
            ldqk(0, 0, 1)
            ldx1(0, 6)
            ldx1(0, 7)
            nc.sync.dma_start(tri_t[:], tri_d[:])
            for ih in range(2):
                wcol(wv_t[:, 4 * ih : 4 * ih + 4, :],
                     wv_d[512 * ih : 512 * ih + 512, :])
            for off in (DH, 0):
                ldqk(1, off)
            ldx(1)
            for off in (DH, 0):
                ldqk(2, off)
            ldx(2)
            for off in (DH, 0):
                ldqk(3, off)
            ldx(3)
            for hf in range(2):
                wcol(wp_t[:, :, 512 * hf : 512 * (hf + 1)],
                     wp_d[:, 512 * hf : 512 * (hf + 1)])
            nc.vector.memset(ones_c[:], 1.0)

            # ---- persistent stores ----
            # K^T / Q^T bf16 per (feat-pair hp, q-window m): [128 feat, 512]
            k_sb = [
                [pk.tile([P, QC], bf16, tag=f"k{j}_{m}", name=f"k{j}_{m}")
                 for m in range(NQC)]
                for j in range(4)
            ]
            q_sb = [
                [pq.tile([P, QC], bf16, tag=f"q{j}_{m}", name=f"q{j}_{m}")
                 for m in range(NQC)]
                for j in range(4)
            ]
            # V' bf16 per k-block: [128 k, 8 heads, 65] (col 64 = ones)
            vp = [pv.tile([P, 8, 65], bf16, tag=f"vp{kb}", name=f"vp{kb}")
                  for kb in range(NKB)]
            # O^T f32r per (feat-pair hp, q-chunk n): [128 feat, 512 q]
            o_sb = [
                [posb.tile([P, QC], f32r, tag=f"o{i}_{m}", name=f"o{i}_{m}")
                 for m in range(NQC)]
                for i in range(4)
            ]

            # ---- filler units (software pipelining) ----
            # each unit is split into two half-thunks (4 matmuls each) so the
            # wedge points get finer-grained PE work
            def u_qk(kind, j, m):
                box = {}
                off = 0 if kind == "q" else DH

                def mm(pp, i):
                    nc.tensor.matmul(
                        pp[:],
                        wqk_t[:, i, off + P * j : off + P * (j + 1)],
                        x_t[i][:, QC * m : QC * (m + 1)],
                        start=(i == 0),
                        stop=(i == NCC - 1),
                    )

                def emit_a():
                    box["pp"] = psX.tile([P, QC], f32, tag="fl",
                                         name=f"{kind}ps{j}_{m}")
                    for i in range(4):
                        mm(box["pp"], i)

                def emit_b():
                    for i in range(4, NCC):
                        mm(box["pp"], i)
                    dst = q_sb if kind == "q" else k_sb
                    nc.vector.tensor_copy(dst[j][m][:], box["pp"][:])
                return emit_a, emit_b

            def u_v(kb):
                box = {}

                def mm(pp, i):
                    nc.tensor.matmul(
                        pp[:],
                        x_t[i][:, P * kb : P * (kb + 1)],
                        wv_t[:, i, :],
                        start=(i == 0),
                        stop=(i == NCC - 1),
                    )

                def emit_a():
                    box["pp"] = psX.tile([P, QC], f32, tag="fl",
                                         name=f"vps{kb}")
                    for i in range(4):
                        mm(box["pp"], i)

                def emit_b():
                    for i in range(4, NCC):
                        mm(box["pp"], i)
                    nc.vector.tensor_copy(
                        vp[kb][:, :, 0:64],
                        box["pp"][:].rearrange("p (h d) -> p h d", d=64),
                    )
                    nc.vector.tensor_copy(vp[kb][:, :, 64:65], ones_c[:])
                return emit_a, emit_b

            def pj_mm(pp, n, j2, i2):
                nc.tensor.matmul(
                    pp[:],
                    wp_t[:, i2, P * j2 : P * (j2 + 1)],
                    o_sb[i2][n][:],
                    start=(i2 == 0),
                    stop=(i2 == 3),
                )

            def pj_out(pp, n, j2, act_copy, gp_dma=False):
                oo = psb.tile([P, QC], f32, tag="oo", bufs=8,
                              name=f"oo{j2}_{n}")
                if act_copy:
                    nc.scalar.copy(oo[:], pp[:])
                else:
                    nc.vector.tensor_copy(oo[:], pp[:])
                if gp_dma:
                    q = nc.sync if j2 % 2 == 0 else nc.gpsimd
                else:
                    q = nc.sync if j2 % 2 == 0 else nc.scalar
                q.dma_start(
                    out_d[P * j2 : P * (j2 + 1), QC * n : QC * (n + 1)],
                    oo[:],
                )

            def u_proj(n, j2, ptag="fl", act_copy=False, gp_dma=False):
                def emit():
                    pool = psS if ptag == "sg" else psX
                    pp = pool.tile([P, QC], f32, tag=ptag, name=f"pj{j2}_{n}")
                    for i2 in range(4):
                        pj_mm(pp, n, j2, i2)
                    pj_out(pp, n, j2, act_copy, gp_dma)
                return emit

            pj3_pp = {}

            def u_proj3_partial(j2):
                def emit():
                    pp = psX.tile([P, QC], f32, tag="fl", name=f"pj{j2}_3")
                    pj3_pp[j2] = pp
                    for i2 in range(3):
                        pj_mm(pp, 3, j2, i2)
                return emit

            def qkv_units(m):
                us = [*u_qk("k", 0, m), *u_qk("q", 0, m)]
                for t in range(4):
                    us += [*u_v(4 * m + t)]
                for j in range(1, 4):
                    us += [*u_qk("k", j, m), *u_qk("q", j, m)]
                return us

            # ---- attention chunks with wedged fillers ----
            # FQ entries are (absolute-deadline-wedge-index, emit_fn).
            # Bresenham pacing spreads units across the phase; the deadline
            # forces a unit out before the attention instruction that reads
            # its output is emitted (program order defines data flow).
            FQ = deque()
            pace = {"W": 0, "seen": 0, "emitted": 0, "points": 1, "units": 0}

            def phase(units, points):
                FQ.extend(units)
                pace["seen"] = 0
                pace["emitted"] = 0
                pace["points"] = max(points, 1)
                pace["units"] = len(FQ)

            def wedge():
                pace["W"] += 1
                pace["seen"] += 1
                while FQ and (
                    (pace["emitted"] - 1) * pace["points"]
                    < pace["seen"] * pace["units"]
                    or FQ[0][0] <= pace["W"]
                ):
                    FQ.popleft()[1]()
                    pace["emitted"] += 1

            def qkv_deadlines(m, base):
                # phase m: 8 heads x (ngrp + 3) wedge points; head-pair hp's
                # first consumer (odd head 2hp+1) sits at position hp
                ngrp_m = 2 * (m + 1)
                span = ngrp_m + 3
                dk = lambda j: base + j * span
                dv = lambda kb: base + 1 + ((kb % 4) // 2)
                us = qkv_units(m)
                dl = [dk(0), dk(0), dv(4 * m), dv(4 * m + 1), dv(4 * m + 2),
                      dv(4 * m + 3), dk(1), dk(1), dk(2), dk(2), dk(3), dk(3)]
                dl2 = [d for d in dl for _ in range(2)]
                return list(zip(dl2, us))

            # ---- prologue: the minimal QKV prefix head 0/1 of chunk 0
            # needs; V and the j>0 feature chunks ride the chunk-0 filler
            # queue so their wv/x waits never block the attention start ----
            prologue = qkv_deadlines(0, 0)
            for _, u in prologue[:4]:
                u()
            rest0 = prologue[4:]

            for n in range(NQC):
                nkb = 4 * (n + 1)
                ngrp = nkb // GS
                points = 8 * (ngrp + 3)
                next_base = pace["W"] + points
                if n == 0:
                    phase(rest0 + qkv_deadlines(1, next_base), points)
                elif n < 3:
                    phase(qkv_deadlines(n + 1, next_base), points)
                else:
                    phase([(1 << 30, u_proj(np_, j2)) for np_ in range(3)
                           for j2 in range(8)]
                          + [(1 << 30, u_proj3_partial(j2)) for j2 in (6, 7)],
                          points)

                # pair-major order so o_sb feature chunks complete early for
                # the projection
                for h in (1, 0, 3, 2, 5, 4, 7, 6):
                    wedge()
                    hp, par = divmod(h, 2)
                    r0 = 64 * par
                    po = psX.tile([P, QC], f32, tag="po", name=f"po{h}_{n}")
                    def emit_av(grp, at):
                        for t_i, kb in enumerate(grp):
                            c0 = P * (kb - 4 * n) if kb >= 4 * n else 0
                            nc.tensor.matmul(
                                po[0:65, c0:QC],
                                vp[kb][:, h, :],
                                at[:, QC * t_i + c0 : QC * (t_i + 1)],
                                start=(kb == 0),
                                stop=(kb >= 4 * n),
                            )

                    # A@V is skewed one group behind the scores so the exp
                    # latency is covered by the next group's score matmuls
                    pend_av = None
                    for gi in range(ngrp):
                        grp = [GS * gi, GS * gi + 1]
                        sg = psS.tile([P, GS * QC], f32, tag="sg",
                                      name=f"sg{h}_{n}_{gi}")
                        for t_i, kb in enumerate(grp):
                            lhs_k = k_sb[hp][kb // 4][
                                r0 : r0 + 64,
                                P * (kb % 4) : P * (kb % 4 + 1),
                            ]
                            c0 = P * (kb - 4 * n) if kb >= 4 * n else 0
                            nc.tensor.matmul(
                                sg[:, QC * t_i + c0 : QC * (t_i + 1)],
                                lhs_k,
                                q_sb[hp][n][r0 : r0 + 64, c0:QC],
                                start=True, stop=True,
                            )
                        at = pa.tile([P, GS * QC], bf16, tag="at",
                                     name=f"at{h}_{n}_{gi}")
                        if gi == ngrp - 1:
                            # last group = diag blocks tt2/tt3: exp only the
                            # causally-needed column ranges
                            nc.scalar.activation(at[:, 256:512],
                                                 sg[:, 256:512], EXP,
                                                 scale=0.125)
                            nc.scalar.activation(at[:, 896:1024],
                                                 sg[:, 896:1024], EXP,
                                                 scale=0.125)
                        else:
                            nc.scalar.activation(at[:], sg[:], EXP,
                                                 scale=0.125)
                        # multiplicative 0/1 causal triangle on the diagonal
                        # 128-col bands (DVE, 4x mode on bf16)
                        for t_i, kb in enumerate(grp):
                            if kb >= 4 * n:
                                c0 = QC * t_i + P * (kb - 4 * n)
                                nc.vector.tensor_tensor(
                                    at[:, c0 : c0 + P], at[:, c0 : c0 + P],
                                    tri_t[:], MULT)
                        if pend_av is not None:
                            emit_av(*pend_av)
                        wedge()
                        pend_av = (grp, at)
                    emit_av(*pend_av)
                    # ---- normalize: sums -> broadcast -> recip -> mult ----
                    # no PE instruction in this chain: DVE copies the PSUM
                    # sums row, Pool broadcasts, DVE recips and multiplies.
                    sums = psb.tile([P, QC], f32, tag="sm", name=f"sm{h}_{n}")
                    nc.vector.tensor_copy(sums[0:1, :], po[64:65, :])
                    wedge()
                    sb = psb.tile([P, QC], f32, tag="bc", name=f"bc{h}_{n}")
                    rr = psb.tile([P, QC], f32, tag="rr", name=f"rr{h}_{n}")
                    nc.gpsimd.partition_broadcast(sb[0:64, :], sums[0:1, :])
                    nc.vector.reciprocal_approx_fast(rr[0:64, :], sb[0:64, :])
                    dst = (o_sb[hp][n][0:64, :] if par == 0
                           else o_sb[hp][n][64:128, :])
                    nc.vector.tensor_tensor(dst, po[0:64, :], rr[0:64, :],
                                            MULT)
                    wedge()

            # drain any leftover fillers, then final projection (alternating
            # PSUM tags: the attention rings are idle now, so 4 units pipeline)
            while FQ:
                FQ.popleft()[1]()
            # two more partials on the now-free score rings fill the window
            # while the last head's normalization chain completes
            for j2, ptg in ((0, "sg"), (1, "po"), (3, "sg")):
                pool = psS if ptg == "sg" else psX
                pp = pool.tile([P, QC], f32, tag=ptg, name=f"pj{j2}_3")
                pj3_pp[j2] = pp
                for i2 in range(3):
                    pj_mm(pp, 3, j2, i2)
            for j2 in (6, 7, 0, 1, 3):
                pj_mm(pj3_pp[j2], 3, j2, 3)
                pj_out(pj3_pp[j2], 3, j2, act_copy=(j2 % 2 == 1))
            for j2 in (2, 4, 5):
                u_proj(3, j2, ptag="fl" if j2 % 2 == 0 else "sg",
                       act_copy=(j2 % 2 == 1))()

    nc.compile()
    return nc


def _get_nc():
    if "nc" not in _CACHE:
        _CACHE["nc"] = _build()
    return _CACHE["nc"]


def _make_tri():
    # multiplicative causal keep-mask for a 128x128 diagonal block of
    # S^T[k, q]: tri[r, c] = 1 where q-col c >= k-row r else 0
    tri = np.zeros((P, P), np.float32)
    for r in range(P):
        tri[r, r:] = 1.0
    return tri.astype(ml_dtypes.bfloat16)


def _in_maps(x, w_qkv, w_proj):
    bf = ml_dtypes.bfloat16
    tri = _make_tri()
    maps = []
    for c in range(8):
        b, hh = divmod(c, 2)
        xT = np.ascontiguousarray(x[b].T).astype(bf)
        qcols = w_qkv[:, DH * hh : DH * hh + DH]
        kcols = w_qkv[:, C + DH * hh : C + DH * hh + DH]
        vcols = w_qkv[:, 2 * C + DH * hh : 2 * C + DH * hh + DH]
        maps.append({
            "xt": xT,
            "wqk": np.concatenate([qcols, kcols], axis=1).astype(bf),
            "wv": np.ascontiguousarray(vcols).astype(bf),
            "wp": np.ascontiguousarray(w_proj[DH * hh : DH * hh + DH, :],
                                       dtype=np.float32),
            "tri": tri,
        })
    return maps


def _run(x, w_qkv, w_proj, trace=False):
    from concourse.bass_utils import run_bass_kernel_spmd

    nc = _get_nc()
    maps = _in_maps(x, w_qkv, w_proj)
    res = run_bass_kernel_spmd(nc, maps, list(range(8)), trace=trace)
    out = np.empty((B, T, C), np.float32)
    for b in range(B):
        out[b] = res.results[2 * b]["outT"].T + res.results[2 * b + 1]["outT"].T
    return out, res


def kernel(**inputs):
    x = np.asarray(inputs["x"], dtype=np.float32)
    w_qkv = np.asarray(inputs["w_qkv"], dtype=np.float32)
    w_proj = np.asarray(inputs["w_proj"], dtype=np.float32)
    out, _ = _run(x, w_qkv, w_proj, trace=False)
    return out
